# revision 18
# baseline (speedup 1.0000x reference)
"""SwiGLU-projected causal MHA (B=4, S=2048, D=1024, H=16) on 8 TRN2 NeuronCores.

Sharding: core c -> (batch b = c//2, head-group g = c%2).  Each core computes
the SwiGLU Q/K/V projections for its 512 output channels (= 8 heads) of its
batch, runs causal attention for those heads, and produces a partial output
projection (contraction over its 512 channels).  The host sums the two
partials per batch and adds the output bias.

Pipeline (v2): V and K projections run as a prologue; the Q projection for
q-tile qg+1 is spliced between the attention head-pairs of q-tile qg so the
PE never idles waiting on softmax exps (and stays at full p-state).

Per-core device layout:
  QT/KT [128p, 4j, n]       channels on partitions (local ch = j*128 + p),
                            seq on free.  Head hl -> chunk hl//2, partition
                            base 64*(hl%2).
  V     [128p, 16nt, 8hl, 65]  seq on partitions; per head 64 channels plus
                            a ones column, so the AV matmul emits the
                            softmax denominator as ctx row 64.
  Scores are computed transposed, S^T [k-part, q-free], in [128,1024] PSUM
  pairs (2 banks) so one scalar-engine exp covers two k-chunks.  Masking is
  multiplicative on diagonal blocks only (on the Pool engine).
  Normalization: reciprocal_approx_fast on the denominator row, a tiny PE
  matmul broadcasts it across 64 partitions, one DVE mult applies it.
"""
import sys

sys.path.insert(0, "/opt/trn_rl_repo")
import numpy as np

import concourse.bacc as bacc
import concourse.tile as tile
import concourse.mybir as mybir

B, S, D = 4, 2048, 1024
H, DK = 16, 64
NCORES = 8
GCH = 512          # channels per core (8 heads)
NT = S // 128      # 16 seq chunks
F32 = mybir.dt.float32
ACTF = mybir.ActivationFunctionType
ALU = mybir.AluOpType

TRACE = False          # set by test.py for profiling runs
TRACE_CORES = None
LAST_RESULT = None     # BassKernelResults stash for test.py
MM_DTYPE = "bf16"
DEBUG_DUMP = False


def build_program(mask_mode):
    """mask_mode: 'causal' (tril), 'full' (all ones), 'general' (arbitrary)."""
    MMD = mybir.dt.bfloat16 if MM_DTYPE == "bf16" else mybir.dt.float32r
    nc = bacc.Bacc("TRN2", target_bir_lowering=False, debug=False)

    xT = {s: nc.dram_tensor(f"x{s}T", [D, S], MMD, kind="ExternalInput")
          for s in "qkv"}
    w1T = {s: nc.dram_tensor(f"w1T_{s}", [D, GCH], MMD, kind="ExternalInput")
           for s in "qkv"}
    w2T = {s: nc.dram_tensor(f"w2T_{s}", [D, GCH], MMD, kind="ExternalInput")
           for s in "qkv"}
    bias_d = {}
    for s in "qk":
        for bn in ("b1", "b2", "b1h"):
            bias_d[f"{bn}_{s}"] = nc.dram_tensor(f"{bn}_{s}", [128, 4], F32,
                                                 kind="ExternalInput")
    b1v_d = nc.dram_tensor("b1_v", [1, GCH], MMD, kind="ExternalInput")
    b2v_d = nc.dram_tensor("b2_v", [1, GCH], MMD, kind="ExternalInput")
    woT_d = nc.dram_tensor("woT", [128, 4, D], MMD, kind="ExternalInput")
    pat_d = m01T_d = None
    if mask_mode == "causal":
        pat_d = nc.dram_tensor("pat", [128, 4, 512], MMD, kind="ExternalInput")
    elif mask_mode == "general":
        m01T_d = nc.dram_tensor("m01T", [S, S], MMD, kind="ExternalInput")
    pout_d = nc.dram_tensor("pout", [S, D], F32, kind="ExternalOutput")
    dbg = {}
    if DEBUG_DUMP:
        MMDSTR = mybir.dt.bfloat16 if MM_DTYPE == "bf16" else mybir.dt.float32r
        dbg["qt"] = nc.dram_tensor("dbg_qt", [4, 128, 4, 512], MMDSTR,
                                   kind="ExternalOutput")
        dbg["kt"] = nc.dram_tensor("dbg_kt", [128, 4, S], MMDSTR,
                                   kind="ExternalOutput")
        dbg["v"] = nc.dram_tensor("dbg_v", [128, NT, 8, 65], MMDSTR,
                                  kind="ExternalOutput")
        dbg["ct"] = nc.dram_tensor("dbg_ct", [4, 128, 4, 512], MMDSTR,
                                   kind="ExternalOutput")
        dbg["rec"] = nc.dram_tensor("dbg_rec", [4, 4, 2, 512], F32,
                                    kind="ExternalOutput")

    def kc_count(qg):
        return 4 * qg + 4 if mask_mode == "causal" else NT

    with tile.TileContext(nc) as tc:
        with (
            tc.tile_pool(name="persist", bufs=1) as persist,
            tc.tile_pool(name="qtpool", bufs=2) as qtpool,
            tc.tile_pool(name="ctpool", bufs=2) as ctpool,
            tc.tile_pool(name="xpool", bufs=10) as xpool,
            tc.tile_pool(name="stage", bufs=8) as stage,
            tc.tile_pool(name="apool", bufs=6) as apool,
            tc.tile_pool(name="npool", bufs=4) as npool,
            tc.tile_pool(name="otpool", bufs=4) as otpool,
            tc.tile_pool(name="mpool", bufs=2) as mpool,
            tc.tile_pool(name="ps", bufs=2, space="PSUM") as ps,
        ):
            # ---- v weights + first x tiles first; constants go on the idle
            # Pool DMA queue so the SP queue reaches the v inputs quickly ----
            wsb = {}

            def load_w(s):
                for wi, wt in (("1", w1T), ("2", w2T)):
                    w = persist.tile([128, 8, GCH], MMD, tag=f"w{wi}{s}",
                                     name=f"w{wi}{s}")
                    nc.sync.dma_start(
                        w[:], wt[s][:].rearrange("(dc p) o -> p dc o", p=128)
                    )
                    wsb[f"{wi}{s}"] = w

            def load_x_tiles(s, t):
                xts = []
                for dc in range(8):
                    xt = xpool.tile([128, 512], MMD, tag="xt", name="xt")
                    nc.sync.dma_start(
                        xt[:],
                        xT[s][dc * 128:(dc + 1) * 128, t * 512:(t + 1) * 512],
                    )
                    xts.append(xt)
                return xts

            load_w("v")
            xts_pro = load_x_tiles("v", 0)

            kt_sb = persist.tile([128, 4, S], MMD, tag="kt")
            v_sb = persist.tile([128, NT, 8, 65], MMD, tag="v")
            woT_sb = persist.tile([128, 4, D], MMD, tag="wo")
            nc.gpsimd.dma_start(woT_sb[:], woT_d[:])

            onesf = persist.tile([1, 128], F32, tag="onesf")
            ones_r = persist.tile([1, 128], MMD, tag="ones_r")
            nc.any.memset(onesf[:], 1.0)
            nc.vector.tensor_copy(ones_r[:], onesf[:])
            onescol = persist.tile([128, 1], F32, tag="onescol")
            nc.any.memset(onescol[:], 1.0)
            nc.vector.tensor_copy(
                v_sb[:, :, :, 64:65],
                onescol[:, None, :].to_broadcast([128, NT, 8, 1]),
            )
            if mask_mode == "causal":
                pat_sb = persist.tile([128, 4, 512], MMD, tag="pat")
                nc.gpsimd.dma_start(pat_sb[:], pat_d[:])

            bias = {}
            for s in "qk":
                for bn in ("b1", "b2", "b1h"):
                    t = persist.tile([128, 4], F32, tag=f"{bn}{s}",
                                     name=f"{bn}{s}")
                    nc.gpsimd.dma_start(t[:], bias_d[f"{bn}_{s}"][:])
                    bias[f"{bn}{s}"] = t
            b1vr = persist.tile([1, GCH], MMD, tag="b1v")
            b2vr = persist.tile([1, GCH], MMD, tag="b2v")
            nc.gpsimd.dma_start(b1vr[:], b1v_d[:])
            nc.gpsimd.dma_start(b2vr[:], b2v_d[:])
            load_w("k")
            load_w("q")

            def proj_unit(s, t, jh, xts, dst):
                """One jh half of a 512-seq projection tile: 32 matmuls plus
                the silu drains for j = 2*jh + {0,1}.  dst[j, 0:512] gets the
                result (qt/kt layout [128, 4, 512] slice, or None for v)."""
                pstiles = [(ps.tile([128, 512], F32, tag="sc", name=f"p1{jj}", bufs=4),
                            ps.tile([128, 512], F32, tag="sc", name=f"p2{jj}", bufs=4))
                           for jj in range(2)]
                for dc in range(8):
                    for jj in range(2):
                        j = jh * 2 + jj
                        ps1 = pstiles[jj][0][:]
                        ps2 = pstiles[jj][1][:]
                        if s == "v":
                            nc.tensor.matmul(
                                ps1, xts[dc][:, j * 128:(j + 1) * 128],
                                wsb["1v"][:, dc, :],
                                start=(dc == 0), stop=False,
                            )
                            nc.tensor.matmul(
                                ps2, xts[dc][:, j * 128:(j + 1) * 128],
                                wsb["2v"][:, dc, :],
                                start=(dc == 0), stop=False,
                            )
                        else:
                            nc.tensor.matmul(
                                ps1, wsb[f"1{s}"][:, dc, j * 128:(j + 1) * 128],
                                xts[dc][:], start=(dc == 0), stop=(dc == 7),
                            )
                            nc.tensor.matmul(
                                ps2, wsb[f"2{s}"][:, dc, j * 128:(j + 1) * 128],
                                xts[dc][:], start=(dc == 0), stop=(dc == 7),
                            )
                for jj in range(2):
                    j = jh * 2 + jj
                    ps1 = pstiles[jj][0][:]
                    ps2 = pstiles[jj][1][:]
                    if s == "v":
                        # fold free-dim biases into the accumulation
                        nc.tensor.matmul(ps1, ones_r[:], b1vr[:],
                                         start=False, stop=True)
                        nc.tensor.matmul(ps2, ones_r[:], b2vr[:],
                                         start=False, stop=True)
                        T = stage.tile([128, 512], MMD, tag="T", name="T")
                        nc.scalar.activation(T[:], ps1, ACTF.Tanh, scale=0.5)
                        A = stage.tile([128, 512], MMD, tag="A", name="A")
                        nc.scalar.activation(A[:], ps1, ACTF.Identity)
                        u = stage.tile([128, 512], MMD, tag="U", name="U")
                        # u = (T + 1) * A = 2*silu(A)   [DVE 4x mode]
                        nc.vector.scalar_tensor_tensor(
                            u[:], T[:], 1.0, A[:], op0=ALU.add, op1=ALU.mult)
                        nt_i = t * 4 + j
                        nc.vector.tensor_tensor(
                            v_sb[:, nt_i, :, 0:64],
                            ps2.rearrange("p (h d) -> p h d", h=8),
                            u[:].rearrange("p (h d) -> p h d", h=8),
                            ALU.mult,
                        )
                    else:
                        b1 = bias[f"b1{s}"][:, j:j + 1]
                        b1h = bias[f"b1h{s}"][:, j:j + 1]
                        b2 = bias[f"b2{s}"][:, j:j + 1]
                        T = stage.tile([128, 512], MMD, tag="T", name="T")
                        nc.scalar.activation(T[:], ps1, ACTF.Tanh,
                                             scale=0.5, bias=b1h)
                        A = stage.tile([128, 512], MMD, tag="A", name="A")
                        nc.scalar.activation(A[:], ps1, ACTF.Identity, bias=b1)
                        u = stage.tile([128, 512], MMD, tag="U", name="U")
                        nc.vector.scalar_tensor_tensor(
                            u[:], T[:], 1.0, A[:], op0=ALU.add, op1=ALU.mult)
                        # dst = (ps2 + b2) * u
                        nc.vector.scalar_tensor_tensor(
                            dst[:, j, :], ps2, b2, u[:],
                            op0=ALU.add, op1=ALU.mult)

            # ---------------- prologue: V then K ----------------
            for s in "vk":
                for t in range(4):
                    xts = load_x_tiles(s, t)
                    for jh in range(2):
                        proj_unit(s, t, jh, xts,
                                  None if s == "v" else kt_sb[:, :, t * 512:(t + 1) * 512])

            # Q for qg=0 (before the main loop)
            qt_tiles = {}
            qt_tiles[0] = qtpool.tile([128, 4, 512], MMD, tag="qt", name="qt0")
            xts = load_x_tiles("q", 0)
            for jh in range(2):
                proj_unit("q", 0, jh, xts, qt_tiles[0])

            def attn_pj(qg, pj, qt_t, ct_qg, mtiles):
                kcmax = kc_count(qg)
                ctx = [ps.tile([128, 512], F32, tag="cx", name=f"ctx{i}")
                       for i in range(2)]
                for kp in range(0, kcmax, 2):
                    for par in range(2):
                        bp = par * 64
                        hl = 2 * pj + par
                        for kk in range(2):
                            kc = kp + kk
                            sc = ps.tile([128, 512], F32, tag="sc", name="sc", bufs=4)
                            nc.tensor.matmul(
                                sc[:],
                                kt_sb[bp:bp + 64, pj,
                                      kc * 128:(kc + 1) * 128],
                                qt_t[bp:bp + 64, pj, :],
                            )
                            attn = apool.tile([128, 512], MMD, tag="at",
                                              name="attn")
                            nc.scalar.activation(attn[:], sc[:], ACTF.Exp)
                            if mask_mode == "causal" and kc >= 4 * qg:
                                nc.vector.tensor_tensor(
                                    attn[:], attn[:],
                                    pat_sb[:, kc - 4 * qg, :], ALU.mult)
                            elif mask_mode == "general":
                                nc.vector.tensor_tensor(
                                    attn[:], attn[:], mtiles[kc], ALU.mult)
                            nc.tensor.matmul(
                                ctx[par][0:65, :],
                                v_sb[:, kc, hl, :],
                                attn[:],
                                start=(kc == 0), stop=(kc == kcmax - 1),
                            )
                # normalize both heads: batched full-precision reciprocal
                # (den rows at partitions 0/32), PE matmul broadcast
                den = npool.tile([33, 512], F32, tag="dn", name="den")
                nc.gpsimd.memset(den[:], 1.0)
                for par in range(2):
                    nc.vector.tensor_copy(
                        den[32 * par:32 * par + 1, :], ctx[par][64:65, :])
                rec = npool.tile([33, 512], MMD, tag="rc", name="rc")
                with nc.allow_low_precision(reason="softmax denom"):
                    nc.vector.reciprocal(rec[:], den[:])
                recb = npool.tile([1, 512], MMD, tag="rb", name="rb")
                nc.vector.tensor_copy(recb[:], rec[32:33, :])
                rec_rows = (rec[0:1, :], recb[:])
                for par in range(2):
                    if DEBUG_DUMP:
                        nc.sync.dma_start(dbg["rec"][qg, pj, par][None],
                                          rec_rows[par])
                    bc_ps = ps.tile([128, 512], F32, tag="po", name="bc")
                    nc.tensor.matmul(bc_ps[0:64, :], ones_r[0:1, 0:64],
                                     rec_rows[par])
                    bc_sb = npool.tile([64, 512], MMD, tag="bc", name="bcs")
                    nc.vector.tensor_copy(bc_sb[:], bc_ps[0:64, :])
                    bp = par * 64
                    nc.vector.tensor_tensor(
                        ct_qg[bp:bp + 64, pj, :],
                        ctx[par][0:64, :], bc_sb[:], ALU.mult)

            def outproj(qg, ct_qg):
                for ns in range(4):
                    nt_i = qg * 4 + ns
                    nsl = slice(ns * 128, (ns + 1) * 128)
                    for oh in range(2):
                        po = ps.tile([128, 512], F32, tag="po", name="po")
                        for j in range(4):
                            nc.tensor.matmul(
                                po[:], ct_qg[:, j, nsl],
                                woT_sb[:, j, oh * 512:(oh + 1) * 512],
                                start=(j == 0), stop=(j == 3),
                            )
                        ot = otpool.tile([128, 512], F32, tag="ot", name="ot")
                        nc.vector.tensor_copy(ot[:], po[:])
                        nc.sync.dma_start(
                            pout_d[nt_i * 128:(nt_i + 1) * 128,
                                   oh * 512:(oh + 1) * 512],
                            ot[:],
                        )

            # ---------------- main loop over q-groups ----------------
            for qg in range(4):
                ct_qg = ctpool.tile([128, 4, 512], MMD, tag="ct", name="ct")
                mtiles = None
                if mask_mode == "general":
                    mtiles = []
                    mt_sb = mpool.tile([128, NT, 512], MMD, tag="mt",
                                       name="mt")
                    for kc in range(kc_count(qg)):
                        nc.sync.dma_start(
                            mt_sb[:, kc, :],
                            m01T_d[kc * 128:(kc + 1) * 128,
                                   qg * 512:(qg + 1) * 512],
                        )
                        mtiles.append(mt_sb[:, kc, :])

                xts_next = None
                for pj in range(4):
                    attn_pj(qg, pj, qt_tiles[qg], ct_qg, mtiles)
                    # splice the next q-tile's projection between head pairs
                    if qg < 3:
                        if pj == 0:
                            qt_tiles[qg + 1] = qtpool.tile(
                                [128, 4, 512], MMD, tag="qt", name="qt")
                            xts_next = load_x_tiles("q", qg + 1)
                            proj_unit("q", qg + 1, 0, xts_next,
                                      qt_tiles[qg + 1])
                        elif pj == 1:
                            proj_unit("q", qg + 1, 1, xts_next,
                                      qt_tiles[qg + 1])
                if DEBUG_DUMP:
                    nc.sync.dma_start(dbg["ct"][qg][:], ct_qg[:])
                    nc.sync.dma_start(dbg["qt"][qg][:], qt_tiles[qg][:])
                outproj(qg, ct_qg)
            if DEBUG_DUMP:
                nc.sync.dma_start(dbg["kt"][:], kt_sb[:])
                nc.sync.dma_start(dbg["v"][:], v_sb[:])
    nc.compile()
    return nc


def _host_prepare(inputs):
    """Split the full problem into 8 per-core input maps + host-side info."""
    q = np.asarray(inputs["query"], dtype=np.float32)
    k = np.asarray(inputs["key"], dtype=np.float32)
    v = np.asarray(inputs["value"], dtype=np.float32)
    mask = np.asarray(inputs["mask"])
    w = {n: np.asarray(inputs[n], dtype=np.float32)
         for n in ("wq1", "wq2", "wk1", "wk2", "wv1", "wv2", "wo")}
    bias = {n: np.asarray(inputs[n], dtype=np.float32)
            for n in ("bq1", "bq2", "bk1", "bk2", "bv1", "bv2", "bo")}

    m = mask.reshape(S, S)
    if np.array_equal(m != 0, np.tril(np.ones((S, S), bool))):
        mask_mode = "causal"
    elif np.all(m != 0):
        mask_mode = "full"
    else:
        mask_mode = "general"

    pat = None
    m01T = None
    if mask_mode == "causal":
        kk = np.arange(128)[:, None]
        qq = np.arange(512)[None, :]
        pat = np.stack(
            [(kk + 128 * i <= qq).astype(np.float32) for i in range(4)], axis=1
        )  # [128, 4, 512]
        pat = np.ascontiguousarray(pat)
    elif mask_mode == "general":
        m01T = np.ascontiguousarray((m != 0).T.astype(np.float32))

    scale = 1.0 / np.sqrt(DK).astype(np.float32)

    if MM_DTYPE == "bf16":
        import ml_dtypes

        mmd_np = ml_dtypes.bfloat16
    else:
        mmd_np = np.float32

    def cvt(a):
        return np.ascontiguousarray(a).astype(mmd_np)

    in_maps = []
    for c in range(NCORES):
        b, g = divmod(c, 2)
        sl = slice(g * GCH, (g + 1) * GCH)
        im = {
            "xqT": cvt(q[b].T),
            "xkT": cvt(k[b].T),
            "xvT": cvt(v[b].T),
            "w1T_q": cvt(w["wq1"][sl].T),
            # fold the 1/sqrt(dk) score scale into the non-silu Q branch,
            # and 0.5 everywhere (silu computed as A*(1+tanh(A/2)) = 2*silu)
            "w2T_q": cvt(w["wq2"][sl].T * (scale * 0.5)),
            "w2T_k": cvt(w["wk2"][sl].T * 0.5),
            "w2T_v": cvt(w["wv2"][sl].T * 0.5),
            "w1T_k": cvt(w["wk1"][sl].T),
            "w1T_v": cvt(w["wv1"][sl].T),
            "b1_q": np.ascontiguousarray(bias["bq1"][sl].reshape(4, 128).T),
            "b1h_q": np.ascontiguousarray(
                (bias["bq1"][sl] * 0.5).reshape(4, 128).T),
            "b2_q": np.ascontiguousarray(
                (bias["bq2"][sl] * (scale * 0.5)).reshape(4, 128).T),
            "b1_k": np.ascontiguousarray(bias["bk1"][sl].reshape(4, 128).T),
            "b1h_k": np.ascontiguousarray(
                (bias["bk1"][sl] * 0.5).reshape(4, 128).T),
            "b2_k": np.ascontiguousarray(
                (bias["bk2"][sl] * 0.5).reshape(4, 128).T),
            "b1_v": cvt(bias["bv1"][sl].reshape(1, GCH)),
            "b2_v": cvt((bias["bv2"][sl] * 0.5).reshape(1, GCH)),
            "woT": cvt(
                w["wo"][:, sl].T.reshape(4, 128, D).transpose(1, 0, 2)),
        }
        if mask_mode == "causal":
            im["pat"] = cvt(pat)
        elif mask_mode == "general":
            im["m01T"] = cvt(m01T)
        in_maps.append(im)
    return mask_mode, in_maps, bias["bo"]


def kernel(**inputs):
    global LAST_RESULT
    mask_mode, in_maps, bo = _host_prepare(inputs)
    nc = build_program(mask_mode)

    import concourse.bass_utils as bu

    if TRACE:
        import types

        try:
            from trn_agent_boot.trn_boot import _ntff_profile_via_ctypes

            hook = _ntff_profile_via_ctypes("/opt/axon/libaxon_pjrt.so")
            m = types.ModuleType("antenv.axon_hooks")
            m.get_axon_ntff_profile_hook = lambda: hook
            import antenv  # noqa: F401

            sys.modules["antenv.axon_hooks"] = m
            bu.upload_artifacts = lambda d: "local://skipped"
        except Exception as e:
            print("profiling hook install failed:", e)

    res = bu.run_bass_kernel_spmd(
        nc, in_maps, core_ids=list(range(NCORES)),
        trace=TRACE, trace_cores=TRACE_CORES,
    )
    LAST_RESULT = res

    out = np.empty((B, S, D), dtype=np.float32)
    for b in range(B):
        out[b] = (res.results[2 * b]["pout"] + res.results[2 * b + 1]["pout"]
                  + bo[None, :])
    return out


# revision 20
# speedup vs baseline: 1.2513x; 1.2513x over previous
"""SwiGLU-projected causal MHA (B=4, S=2048, D=1024, H=16) on 8 TRN2 NeuronCores.

Baseline (572888 ns) restored from the original staged kernel.

Sharding: core c -> (batch b = c//2, head-group g = c%2).  Each core computes
the SwiGLU Q/K/V projections for its 512 output channels (= 8 heads) of its
batch, runs causal attention for those heads, and produces a partial output
projection (contraction over its 512 channels).  The host sums the two
partials per batch and adds the output bias.
"""
import sys

sys.path.insert(0, "/opt/trn_rl_repo")
import numpy as np

import concourse.bacc as bacc
import concourse.tile as tile
import concourse.mybir as mybir

B, S, D = 4, 2048, 1024
H, DK = 16, 64
NCORES = 8
GCH = 512          # channels per core (8 heads)
NT = S // 128      # 16 seq chunks
F32 = mybir.dt.float32
F32R = mybir.dt.float32r
ACTF = mybir.ActivationFunctionType
ALU = mybir.AluOpType

TRACE = False          # set by test.py for profiling runs
TRACE_CORES = None
LAST_RESULT = None     # BassKernelResults stash for test.py
MM_DTYPE = "bf16"      # "bf16" (fast weight load) or "f32r" (higher precision)


def build_program(mask_mode):
    """mask_mode: 'causal' (tril), 'full' (all ones), 'general' (arbitrary)."""
    MMD = mybir.dt.bfloat16 if MM_DTYPE == "bf16" else F32R
    nc = bacc.Bacc("TRN2", target_bir_lowering=False, debug=False)

    xT = {s: nc.dram_tensor(f"x{s}T", [D, S], MMD, kind="ExternalInput")
          for s in "qkv"}
    w1T = {s: nc.dram_tensor(f"w1T_{s}", [D, GCH], MMD, kind="ExternalInput")
           for s in "qkv"}
    w2T = {s: nc.dram_tensor(f"w2T_{s}", [D, GCH], MMD, kind="ExternalInput")
           for s in "qkv"}
    bias_d = {}
    for s in "qk":
        for bn in ("b1", "b2", "b1h"):
            bias_d[f"{bn}_{s}"] = nc.dram_tensor(f"{bn}_{s}", [128, 4], F32,
                                                 kind="ExternalInput")
    b1v_d = nc.dram_tensor("b1_v", [1, GCH], MMD, kind="ExternalInput")
    b2v_d = nc.dram_tensor("b2_v", [1, GCH], MMD, kind="ExternalInput")
    woT_d = nc.dram_tensor("woT", [128, 4, D], MMD, kind="ExternalInput")
    pat_d = m01T_d = None
    if mask_mode == "causal":
        pat_d = nc.dram_tensor("pat", [128, 4, 512], MMD, kind="ExternalInput")
    elif mask_mode == "general":
        m01T_d = nc.dram_tensor("m01T", [S, S], MMD, kind="ExternalInput")
    pout_d = nc.dram_tensor("pout", [S, D], F32, kind="ExternalOutput")

    def kc_count(qg):
        return 4 * qg + 4 if mask_mode == "causal" else NT

    with tile.TileContext(nc) as tc:
        with (
            tc.tile_pool(name="persist", bufs=1) as persist,
        ):
            qt_sb = persist.tile([128, 4, S], MMD, tag="qt")
            kt_sb = persist.tile([128, 4, S], MMD, tag="kt")
            v_sb = persist.tile([128, NT, 8, 65], MMD, tag="v")
            woT_sb = persist.tile([128, 4, D], MMD, tag="wo")
            nc.sync.dma_start(woT_sb[:], woT_d[:])
            onesf = persist.tile([1, 128], F32, tag="onesf")
            ones_r = persist.tile([1, 128], MMD, tag="ones_r")
            nc.any.memset(onesf[:], 1.0)
            nc.vector.tensor_copy(ones_r[:], onesf[:])
            onescol = persist.tile([128, 1], F32, tag="onescol")
            nc.any.memset(onescol[:], 1.0)
            nc.vector.tensor_copy(
                v_sb[:, :, :, 64:65],
                onescol[:, None, :].to_broadcast([128, NT, 8, 1]),
            )
            if mask_mode == "causal":
                pat_sb = persist.tile([128, 4, 512], MMD, tag="pat")
                nc.sync.dma_start(pat_sb[:], pat_d[:])

            # ---------------- Phase A: SwiGLU projections ----------------
            with (
                tc.tile_pool(name="wpool", bufs=2) as wpool,
                tc.tile_pool(name="xpool", bufs=9) as xpool,
                tc.tile_pool(name="stage", bufs=4) as stage,
                tc.tile_pool(name="pps", bufs=6, space="PSUM") as pps,
            ):
                for s in "vkq":
                    w1sb = wpool.tile([128, 8, GCH], MMD, tag="w")
                    w2sb = wpool.tile([128, 8, GCH], MMD, tag="w")
                    nc.sync.dma_start(
                        w1sb[:], w1T[s][:].rearrange("(dc p) o -> p dc o", p=128)
                    )
                    nc.sync.dma_start(
                        w2sb[:], w2T[s][:].rearrange("(dc p) o -> p dc o", p=128)
                    )
                    if s != "v":
                        b1sb = persist.tile([128, 4], F32, tag=f"b1{s}")
                        b2sb = persist.tile([128, 4], F32, tag=f"b2{s}")
                        b1hsb = persist.tile([128, 4], F32, tag=f"b1h{s}")
                        nc.sync.dma_start(b1sb[:], bias_d[f"b1_{s}"][:])
                        nc.sync.dma_start(b2sb[:], bias_d[f"b2_{s}"][:])
                        nc.sync.dma_start(b1hsb[:], bias_d[f"b1h_{s}"][:])
                    else:
                        b1vr = persist.tile([1, GCH], MMD, tag="b1v")
                        b2vr = persist.tile([1, GCH], MMD, tag="b2v")
                        nc.sync.dma_start(b1vr[:], b1v_d[:])
                        nc.sync.dma_start(b2vr[:], b2v_d[:])

                    for t in range(4):  # 512-wide seq tiles
                        xts = []
                        for dc in range(8):
                            xt = xpool.tile([128, 512], MMD, tag="xt")
                            nc.sync.dma_start(
                                xt[:],
                                xT[s][dc * 128:(dc + 1) * 128,
                                      t * 512:(t + 1) * 512],
                            )
                            xts.append(xt)
                        for jh in range(2):
                            ps1 = [pps.tile([128, 512], F32, tag="pp",
                                            name=f"ps1_{i}")
                                   for i in range(2)]
                            ps2 = [pps.tile([128, 512], F32, tag="pp",
                                            name=f"ps2_{i}")
                                   for i in range(2)]
                            for dc in range(8):
                                for jj in range(2):
                                    j = jh * 2 + jj
                                    if s == "v":
                                        # seq on partitions: lhsT = x chunk
                                        nc.tensor.matmul(
                                            ps1[jj][:],
                                            xts[dc][:, j * 128:(j + 1) * 128],
                                            w1sb[:, dc, :],
                                            start=(dc == 0), stop=False,
                                        )
                                        nc.tensor.matmul(
                                            ps2[jj][:],
                                            xts[dc][:, j * 128:(j + 1) * 128],
                                            w2sb[:, dc, :],
                                            start=(dc == 0), stop=False,
                                        )
                                    else:
                                        # channels on partitions: lhsT = w chunk
                                        nc.tensor.matmul(
                                            ps1[jj][:],
                                            w1sb[:, dc, j * 128:(j + 1) * 128],
                                            xts[dc][:],
                                            start=(dc == 0), stop=(dc == 7),
                                        )
                                        nc.tensor.matmul(
                                            ps2[jj][:],
                                            w2sb[:, dc, j * 128:(j + 1) * 128],
                                            xts[dc][:],
                                            start=(dc == 0), stop=(dc == 7),
                                        )
                            for jj in range(2):
                                j = jh * 2 + jj
                                act = stage.tile([128, 512], F32, tag="act")
                                if s == "v":
                                    # fold the biases into the accumulation
                                    # (they vary along the free/channel dim)
                                    nc.tensor.matmul(
                                        ps1[jj][:], ones_r[:], b1vr[:],
                                        start=False, stop=True,
                                    )
                                    nc.tensor.matmul(
                                        ps2[jj][:], ones_r[:], b2vr[:],
                                        start=False, stop=True,
                                    )
                                    nc.scalar.activation(
                                        act[:], ps1[jj][:], ACTF.Tanh,
                                        scale=0.5,
                                    )
                                    u = stage.tile([128, 512], F32, tag="u")
                                    nc.vector.tensor_tensor(
                                        u[:], ps1[jj][:], act[:], ALU.mult
                                    )
                                    nc.vector.tensor_tensor(
                                        act[:], ps1[jj][:], u[:], ALU.add
                                    )
                                    nt_i = t * 4 + j
                                    nc.vector.tensor_tensor(
                                        v_sb[:, nt_i, :, 0:64],
                                        ps2[jj][:].rearrange(
                                            "p (h d) -> p h d", h=8
                                        ),
                                        act[:].rearrange(
                                            "p (h d) -> p h d", h=8
                                        ),
                                        ALU.mult,
                                    )
                                else:
                                    bias1 = b1sb[:, j:j + 1]
                                    bias2 = b2sb[:, j:j + 1]
                                    # act = tanh((A)/2), A = ps1 + b1
                                    nc.scalar.activation(
                                        act[:], ps1[jj][:], ACTF.Tanh,
                                        scale=0.5, bias=b1hsb[:, j:j + 1],
                                    )
                                    a_sb = stage.tile([128, 512], F32,
                                                      tag="u")
                                    nc.vector.tensor_scalar_add(
                                        a_sb[:], ps1[jj][:], bias1
                                    )
                                    # act = A*(1+tanh(A/2)) = 2*silu(A)
                                    nc.vector.scalar_tensor_tensor(
                                        act[:], act[:], 1.0, a_sb[:],
                                        op0=ALU.add, op1=ALU.mult,
                                    )
                                    dst = (qt_sb if s == "q" else kt_sb)[
                                        :, j, t * 512:(t + 1) * 512
                                    ]
                                    nc.vector.scalar_tensor_tensor(
                                        dst, ps2[jj][:], bias2, act[:],
                                        op0=ALU.add, op1=ALU.mult,
                                    )

            # ------------- Phase B+C: attention + output projection -------
            with (
                tc.tile_pool(name="scps", bufs=4, space="PSUM") as scps,
                tc.tile_pool(name="cxps", bufs=2, space="PSUM") as cxps,
                tc.tile_pool(name="bcps", bufs=1, space="PSUM") as bcps,
                tc.tile_pool(name="apool", bufs=6) as apool,
                tc.tile_pool(name="ctpool", bufs=2) as ctpool,
                tc.tile_pool(name="smalls", bufs=4) as smalls,
                tc.tile_pool(name="ostage", bufs=4) as ostage,
                tc.tile_pool(name="mpool", bufs=2) as mpool,
            ):
                for qg in range(4):
                    kcmax = kc_count(qg)
                    qsl = slice(qg * 512, (qg + 1) * 512)
                    ct_qg = ctpool.tile([128, 4, 512], MMD, tag="ct")

                    mtiles = None
                    if mask_mode == "general":
                        mtiles = []
                        mt_sb = mpool.tile([128, NT, 512], MMD, tag="mt")
                        for kc in range(kcmax):
                            nc.sync.dma_start(
                                mt_sb[:, kc, :],
                                m01T_d[kc * 128:(kc + 1) * 128, qsl],
                            )
                            mtiles.append(mt_sb[:, kc, :])

                    for pj in range(4):   # head pair: hl = 2*pj (+1)
                        ctx = [cxps.tile([128, 512], F32, tag="cx",
                                         name=f"ctx_{i}")
                               for i in range(2)]
                        for kc in range(kcmax):
                            ksl = slice(kc * 128, (kc + 1) * 128)
                            sc = [scps.tile([128, 512], F32, tag="sc",
                                            name=f"sc_{i}")
                                  for i in range(2)]
                            for par in range(2):
                                bp = par * 64
                                nc.tensor.matmul(
                                    sc[par][:],
                                    kt_sb[bp:bp + 64, pj, ksl],
                                    qt_sb[bp:bp + 64, pj, qsl],
                                )
                            for par in range(2):
                                attn = apool.tile([128, 512], MMD, tag="at")
                                nc.scalar.activation(
                                    attn[:], sc[par][:], ACTF.Exp
                                )
                                if mask_mode == "causal" and kc >= 4 * qg:
                                    nc.vector.tensor_tensor(
                                        attn[:], attn[:],
                                        pat_sb[:, kc - 4 * qg, :],
                                        ALU.mult,
                                    )
                                elif mask_mode == "general":
                                    nc.vector.tensor_tensor(
                                        attn[:], attn[:],
                                        mtiles[kc], ALU.mult,
                                    )
                                hl = 2 * pj + par
                                nc.tensor.matmul(
                                    ctx[par][0:65, :],
                                    v_sb[:, kc, hl, :],
                                    attn[:],
                                    start=(kc == 0),
                                    stop=(kc == kcmax - 1),
                                )
                        # normalize both heads of the pair into ct_qg.
                        # One batched reciprocal; denominator rows live at
                        # partitions 0 and 32 (the only legal operand bases).
                        den = smalls.tile([33, 512], F32, tag="den")
                        nc.gpsimd.memset(den[:], 1.0)
                        for par in range(2):
                            nc.vector.tensor_copy(
                                den[32 * par:32 * par + 1, :],
                                ctx[par][64:65, :],
                            )
                        rec = smalls.tile([33, 512], MMD, tag="rec")
                        with nc.allow_low_precision(reason="f32r==fp32"):
                            nc.vector.reciprocal(rec[:], den[:])
                        recb = smalls.tile([1, 512], MMD, tag="recb")
                        nc.vector.tensor_copy(recb[:], rec[32:33, :])
                        rec_rows = (rec[0:1, :], recb[:])
                        for par in range(2):
                            bc_ps = bcps.tile([128, 512], F32, tag="bc")
                            nc.tensor.matmul(
                                bc_ps[0:64, :], ones_r[0:1, 0:64],
                                rec_rows[par],
                            )
                            bc_sb = smalls.tile([64, 512], F32, tag="bcs")
                            nc.vector.tensor_copy(bc_sb[:], bc_ps[0:64, :])
                            bp = par * 64
                            nc.vector.tensor_tensor(
                                ct_qg[bp:bp + 64, pj, :],
                                ctx[par][0:64, :], bc_sb[:], ALU.mult,
                            )

                    # ---- output projection for this q-group ----
                    for ns in range(4):
                        nt_i = qg * 4 + ns
                        nsl = slice(ns * 128, (ns + 1) * 128)
                        for oh in range(2):
                            po = bcps.tile([128, 512], F32, tag="bc")
                            for j in range(4):
                                nc.tensor.matmul(
                                    po[:],
                                    ct_qg[:, j, nsl],
                                    woT_sb[:, j, oh * 512:(oh + 1) * 512],
                                    start=(j == 0), stop=(j == 3),
                                )
                            ot = ostage.tile([128, 512], F32, tag="ot")
                            nc.vector.tensor_copy(ot[:], po[:])
                            nc.sync.dma_start(
                                pout_d[nt_i * 128:(nt_i + 1) * 128,
                                       oh * 512:(oh + 1) * 512],
                                ot[:],
                            )
    nc.compile()
    return nc


def _host_prepare(inputs):
    """Split the full problem into 8 per-core input maps + host-side info."""
    q = np.asarray(inputs["query"], dtype=np.float32)
    k = np.asarray(inputs["key"], dtype=np.float32)
    v = np.asarray(inputs["value"], dtype=np.float32)
    mask = np.asarray(inputs["mask"])
    w = {n: np.asarray(inputs[n], dtype=np.float32)
         for n in ("wq1", "wq2", "wk1", "wk2", "wv1", "wv2", "wo")}
    bias = {n: np.asarray(inputs[n], dtype=np.float32)
            for n in ("bq1", "bq2", "bk1", "bk2", "bv1", "bv2", "bo")}

    m = mask.reshape(S, S)
    if np.array_equal(m != 0, np.tril(np.ones((S, S), bool))):
        mask_mode = "causal"
    elif np.all(m != 0):
        mask_mode = "full"
    else:
        mask_mode = "general"

    pat = None
    m01T = None
    if mask_mode == "causal":
        kk = np.arange(128)[:, None]
        qq = np.arange(512)[None, :]
        pat = np.stack(
            [(kk + 128 * i <= qq).astype(np.float32) for i in range(4)], axis=1
        )  # [128, 4, 512]
        pat = np.ascontiguousarray(pat)
    elif mask_mode == "general":
        m01T = np.ascontiguousarray((m != 0).T.astype(np.float32))

    scale = 1.0 / np.sqrt(DK).astype(np.float32)

    if MM_DTYPE == "bf16":
        import ml_dtypes

        mmd_np = ml_dtypes.bfloat16
    else:
        mmd_np = np.float32

    def cvt(a):
        return np.ascontiguousarray(a).astype(mmd_np)

    in_maps = []
    for c in range(NCORES):
        b, g = divmod(c, 2)
        sl = slice(g * GCH, (g + 1) * GCH)
        im = {
            "xqT": cvt(q[b].T),
            "xkT": cvt(k[b].T),
            "xvT": cvt(v[b].T),
            "w1T_q": cvt(w["wq1"][sl].T),
            # fold the 1/sqrt(dk) score scale into the non-silu Q branch,
            # and 0.5 everywhere (silu computed as A*(1+tanh(A/2)) = 2*silu)
            "w2T_q": cvt(w["wq2"][sl].T * (scale * 0.5)),
            "w2T_k": cvt(w["wk2"][sl].T * 0.5),
            "w2T_v": cvt(w["wv2"][sl].T * 0.5),
            "w1T_k": cvt(w["wk1"][sl].T),
            "w1T_v": cvt(w["wv1"][sl].T),
            "b1_q": np.ascontiguousarray(bias["bq1"][sl].reshape(4, 128).T),
            "b1h_q": np.ascontiguousarray(
                (bias["bq1"][sl] * 0.5).reshape(4, 128).T),
            "b2_q": np.ascontiguousarray(
                (bias["bq2"][sl] * (scale * 0.5)).reshape(4, 128).T),
            "b1_k": np.ascontiguousarray(bias["bk1"][sl].reshape(4, 128).T),
            "b1h_k": np.ascontiguousarray(
                (bias["bk1"][sl] * 0.5).reshape(4, 128).T),
            "b2_k": np.ascontiguousarray(
                (bias["bk2"][sl] * 0.5).reshape(4, 128).T),
            "b1_v": cvt(bias["bv1"][sl].reshape(1, GCH)),
            "b2_v": cvt((bias["bv2"][sl] * 0.5).reshape(1, GCH)),
            "woT": cvt(
                w["wo"][:, sl].T.reshape(4, 128, D).transpose(1, 0, 2)),
        }
        if mask_mode == "causal":
            im["pat"] = cvt(pat)
        elif mask_mode == "general":
            im["m01T"] = cvt(m01T)
        in_maps.append(im)
    return mask_mode, in_maps, bias["bo"]


def kernel(**inputs):
    global LAST_RESULT
    mask_mode, in_maps, bo = _host_prepare(inputs)
    nc = build_program(mask_mode)

    import concourse.bass_utils as bu

    if TRACE:
        import types

        try:
            from trn_agent_boot.trn_boot import _ntff_profile_via_ctypes

            hook = _ntff_profile_via_ctypes("/opt/axon/libaxon_pjrt.so")
            m = types.ModuleType("antenv.axon_hooks")
            m.get_axon_ntff_profile_hook = lambda: hook
            import antenv  # noqa: F401

            sys.modules["antenv.axon_hooks"] = m
            bu.upload_artifacts = lambda d: "local://skipped"
        except Exception as e:
            print("profiling hook install failed:", e)

    res = bu.run_bass_kernel_spmd(
        nc, in_maps, core_ids=list(range(NCORES)),
        trace=TRACE, trace_cores=TRACE_CORES,
    )
    LAST_RESULT = res

    out = np.empty((B, S, D), dtype=np.float32)
    for b in range(B):
        out[b] = (res.results[2 * b]["pout"] + res.results[2 * b + 1]["pout"]
                  + bo[None, :])
    return out


# revision 21
# speedup vs baseline: 1.2583x; 1.0056x over previous
"""SwiGLU-projected causal MHA (B=4, S=2048, D=1024, H=16) on 8 TRN2 NeuronCores.

Baseline (572888 ns) restored from the original staged kernel.

Sharding: core c -> (batch b = c//2, head-group g = c%2).  Each core computes
the SwiGLU Q/K/V projections for its 512 output channels (= 8 heads) of its
batch, runs causal attention for those heads, and produces a partial output
projection (contraction over its 512 channels).  The host sums the two
partials per batch and adds the output bias.
"""
import sys

sys.path.insert(0, "/opt/trn_rl_repo")
import numpy as np

import concourse.bacc as bacc
import concourse.tile as tile
import concourse.mybir as mybir

B, S, D = 4, 2048, 1024
H, DK = 16, 64
NCORES = 8
GCH = 512          # channels per core (8 heads)
NT = S // 128      # 16 seq chunks
F32 = mybir.dt.float32
F32R = mybir.dt.float32r
ACTF = mybir.ActivationFunctionType
ALU = mybir.AluOpType

TRACE = False          # set by test.py for profiling runs
TRACE_CORES = None
LAST_RESULT = None     # BassKernelResults stash for test.py
MM_DTYPE = "bf16"      # "bf16" (fast weight load) or "f32r" (higher precision)


def build_program(mask_mode):
    """mask_mode: 'causal' (tril), 'full' (all ones), 'general' (arbitrary)."""
    MMD = mybir.dt.bfloat16 if MM_DTYPE == "bf16" else F32R
    nc = bacc.Bacc("TRN2", target_bir_lowering=False, debug=False)

    xT = {s: nc.dram_tensor(f"x{s}T", [D, S], MMD, kind="ExternalInput")
          for s in "qkv"}
    w1T = {s: nc.dram_tensor(f"w1T_{s}", [D, GCH], MMD, kind="ExternalInput")
           for s in "qkv"}
    w2T = {s: nc.dram_tensor(f"w2T_{s}", [D, GCH], MMD, kind="ExternalInput")
           for s in "qkv"}
    bias_d = {}
    for s in "qk":
        for bn in ("b1", "b2", "b1h"):
            bias_d[f"{bn}_{s}"] = nc.dram_tensor(f"{bn}_{s}", [128, 4], F32,
                                                 kind="ExternalInput")
    b1v_d = nc.dram_tensor("b1_v", [1, GCH], MMD, kind="ExternalInput")
    b2v_d = nc.dram_tensor("b2_v", [1, GCH], MMD, kind="ExternalInput")
    woT_d = nc.dram_tensor("woT", [128, 4, D], MMD, kind="ExternalInput")
    pat_d = m01T_d = None
    if mask_mode == "causal":
        pat_d = nc.dram_tensor("pat", [128, 4, 512], MMD, kind="ExternalInput")
    elif mask_mode == "general":
        m01T_d = nc.dram_tensor("m01T", [S, S], MMD, kind="ExternalInput")
    pout_d = nc.dram_tensor("pout", [S, D], F32, kind="ExternalOutput")

    def kc_count(qg):
        return 4 * qg + 4 if mask_mode == "causal" else NT

    with tile.TileContext(nc) as tc:
        with (
            tc.tile_pool(name="persist", bufs=1) as persist,
        ):
            qt_sb = persist.tile([128, 4, S], MMD, tag="qt")
            kt_sb = persist.tile([128, 4, S], MMD, tag="kt")
            v_sb = persist.tile([128, NT, 8, 65], MMD, tag="v")
            woT_sb = persist.tile([128, 4, D], MMD, tag="wo")
            nc.sync.dma_start(woT_sb[:], woT_d[:])
            onesf = persist.tile([1, 128], F32, tag="onesf")
            ones_r = persist.tile([1, 128], MMD, tag="ones_r")
            nc.any.memset(onesf[:], 1.0)
            nc.vector.tensor_copy(ones_r[:], onesf[:])
            onescol = persist.tile([128, 1], F32, tag="onescol")
            nc.any.memset(onescol[:], 1.0)
            nc.vector.tensor_copy(
                v_sb[:, :, :, 64:65],
                onescol[:, None, :].to_broadcast([128, NT, 8, 1]),
            )
            if mask_mode == "causal":
                pat_sb = persist.tile([128, 4, 512], MMD, tag="pat")
                nc.sync.dma_start(pat_sb[:], pat_d[:])

            # ---------------- Phase A: SwiGLU projections ----------------
            with (
                tc.tile_pool(name="wpool", bufs=4) as wpool,
                tc.tile_pool(name="xpool", bufs=12) as xpool,
                tc.tile_pool(name="stage", bufs=4) as stage,
                tc.tile_pool(name="pps", bufs=6, space="PSUM") as pps,
            ):
                for s in "vkq":
                    w1sb = wpool.tile([128, 8, GCH], MMD, tag="w")
                    w2sb = wpool.tile([128, 8, GCH], MMD, tag="w")
                    nc.sync.dma_start(
                        w1sb[:], w1T[s][:].rearrange("(dc p) o -> p dc o", p=128)
                    )
                    nc.sync.dma_start(
                        w2sb[:], w2T[s][:].rearrange("(dc p) o -> p dc o", p=128)
                    )
                    if s != "v":
                        b1sb = persist.tile([128, 4], F32, tag=f"b1{s}")
                        b2sb = persist.tile([128, 4], F32, tag=f"b2{s}")
                        b1hsb = persist.tile([128, 4], F32, tag=f"b1h{s}")
                        nc.sync.dma_start(b1sb[:], bias_d[f"b1_{s}"][:])
                        nc.sync.dma_start(b2sb[:], bias_d[f"b2_{s}"][:])
                        nc.sync.dma_start(b1hsb[:], bias_d[f"b1h_{s}"][:])
                    else:
                        b1vr = persist.tile([1, GCH], MMD, tag="b1v")
                        b2vr = persist.tile([1, GCH], MMD, tag="b2v")
                        nc.sync.dma_start(b1vr[:], b1v_d[:])
                        nc.sync.dma_start(b2vr[:], b2v_d[:])

                    for t in range(4):  # 512-wide seq tiles
                        xts = []
                        for dc in range(8):
                            xt = xpool.tile([128, 512], MMD, tag="xt")
                            nc.sync.dma_start(
                                xt[:],
                                xT[s][dc * 128:(dc + 1) * 128,
                                      t * 512:(t + 1) * 512],
                            )
                            xts.append(xt)
                        for jh in range(2):
                            ps1 = [pps.tile([128, 512], F32, tag="pp",
                                            name=f"ps1_{i}")
                                   for i in range(2)]
                            ps2 = [pps.tile([128, 512], F32, tag="pp",
                                            name=f"ps2_{i}")
                                   for i in range(2)]
                            for dc in range(8):
                                for jj in range(2):
                                    j = jh * 2 + jj
                                    if s == "v":
                                        # seq on partitions: lhsT = x chunk
                                        nc.tensor.matmul(
                                            ps1[jj][:],
                                            xts[dc][:, j * 128:(j + 1) * 128],
                                            w1sb[:, dc, :],
                                            start=(dc == 0), stop=False,
                                        )
                                        nc.tensor.matmul(
                                            ps2[jj][:],
                                            xts[dc][:, j * 128:(j + 1) * 128],
                                            w2sb[:, dc, :],
                                            start=(dc == 0), stop=False,
                                        )
                                    else:
                                        # channels on partitions: lhsT = w chunk
                                        nc.tensor.matmul(
                                            ps1[jj][:],
                                            w1sb[:, dc, j * 128:(j + 1) * 128],
                                            xts[dc][:],
                                            start=(dc == 0), stop=(dc == 7),
                                        )
                                        nc.tensor.matmul(
                                            ps2[jj][:],
                                            w2sb[:, dc, j * 128:(j + 1) * 128],
                                            xts[dc][:],
                                            start=(dc == 0), stop=(dc == 7),
                                        )
                            for jj in range(2):
                                j = jh * 2 + jj
                                act = stage.tile([128, 512], F32, tag="act")
                                if s == "v":
                                    # fold the biases into the accumulation
                                    # (they vary along the free/channel dim)
                                    nc.tensor.matmul(
                                        ps1[jj][:], ones_r[:], b1vr[:],
                                        start=False, stop=True,
                                    )
                                    nc.tensor.matmul(
                                        ps2[jj][:], ones_r[:], b2vr[:],
                                        start=False, stop=True,
                                    )
                                    nc.scalar.activation(
                                        act[:], ps1[jj][:], ACTF.Tanh,
                                        scale=0.5,
                                    )
                                    u = stage.tile([128, 512], F32, tag="u")
                                    nc.vector.tensor_tensor(
                                        u[:], ps1[jj][:], act[:], ALU.mult
                                    )
                                    nc.vector.tensor_tensor(
                                        act[:], ps1[jj][:], u[:], ALU.add
                                    )
                                    nt_i = t * 4 + j
                                    nc.vector.tensor_tensor(
                                        v_sb[:, nt_i, :, 0:64],
                                        ps2[jj][:].rearrange(
                                            "p (h d) -> p h d", h=8
                                        ),
                                        act[:].rearrange(
                                            "p (h d) -> p h d", h=8
                                        ),
                                        ALU.mult,
                                    )
                                else:
                                    bias1 = b1sb[:, j:j + 1]
                                    bias2 = b2sb[:, j:j + 1]
                                    # act = tanh((A)/2), A = ps1 + b1
                                    nc.scalar.activation(
                                        act[:], ps1[jj][:], ACTF.Tanh,
                                        scale=0.5, bias=b1hsb[:, j:j + 1],
                                    )
                                    a_sb = stage.tile([128, 512], F32,
                                                      tag="u")
                                    nc.vector.tensor_scalar_add(
                                        a_sb[:], ps1[jj][:], bias1
                                    )
                                    # act = A*(1+tanh(A/2)) = 2*silu(A)
                                    nc.vector.scalar_tensor_tensor(
                                        act[:], act[:], 1.0, a_sb[:],
                                        op0=ALU.add, op1=ALU.mult,
                                    )
                                    dst = (qt_sb if s == "q" else kt_sb)[
                                        :, j, t * 512:(t + 1) * 512
                                    ]
                                    nc.vector.scalar_tensor_tensor(
                                        dst, ps2[jj][:], bias2, act[:],
                                        op0=ALU.add, op1=ALU.mult,
                                    )

            # ------------- Phase B+C: attention + output projection -------
            with (
                tc.tile_pool(name="scps", bufs=2, space="PSUM") as scps,
                tc.tile_pool(name="cxps", bufs=3, space="PSUM") as cxps,
                tc.tile_pool(name="bcps", bufs=1, space="PSUM") as bcps,
                tc.tile_pool(name="apool", bufs=6) as apool,
                tc.tile_pool(name="ctpool", bufs=2) as ctpool,
                tc.tile_pool(name="smalls", bufs=4) as smalls,
                tc.tile_pool(name="ostage", bufs=4) as ostage,
                tc.tile_pool(name="mpool", bufs=2) as mpool,
            ):
                for qg in range(4):
                    kcmax = kc_count(qg)
                    qsl = slice(qg * 512, (qg + 1) * 512)
                    ct_qg = ctpool.tile([128, 4, 512], MMD, tag="ct")

                    mtiles = None
                    if mask_mode == "general":
                        mtiles = []
                        mt_sb = mpool.tile([128, NT, 512], MMD, tag="mt")
                        for kc in range(kcmax):
                            nc.sync.dma_start(
                                mt_sb[:, kc, :],
                                m01T_d[kc * 128:(kc + 1) * 128, qsl],
                            )
                            mtiles.append(mt_sb[:, kc, :])

                    for pj in range(4):   # head pair: hl = 2*pj (+1)
                        ctx = [cxps.tile([128, 512], F32, tag="cx",
                                         name=f"ctx_{i}")
                               for i in range(2)]
                        for kp in range(0, kcmax, 2):
                            sc2 = [scps.tile([128, 1024], F32, tag="sc",
                                             name=f"sc_{i}")
                                   for i in range(2)]
                            for par in range(2):
                                bp = par * 64
                                for kk in range(2):
                                    kc = kp + kk
                                    nc.tensor.matmul(
                                        sc2[par][:, kk * 512:(kk + 1) * 512],
                                        kt_sb[bp:bp + 64, pj,
                                              kc * 128:(kc + 1) * 128],
                                        qt_sb[bp:bp + 64, pj, qsl],
                                    )
                            for par in range(2):
                                attn = apool.tile([128, 1024], MMD, tag="at")
                                nc.scalar.activation(
                                    attn[:], sc2[par][:], ACTF.Exp
                                )
                                if mask_mode == "causal" and kp >= 4 * qg:
                                    i0 = kp - 4 * qg
                                    nc.vector.tensor_tensor(
                                        attn[:], attn[:],
                                        pat_sb[:, i0:i0 + 2, :].rearrange(
                                            "p a b -> p (a b)"),
                                        ALU.mult,
                                    )
                                elif mask_mode == "general":
                                    for kk in range(2):
                                        asl = attn[:, kk * 512:(kk + 1) * 512]
                                        nc.vector.tensor_tensor(
                                            asl, asl, mtiles[kp + kk],
                                            ALU.mult,
                                        )
                                hl = 2 * pj + par
                                for kk in range(2):
                                    kc = kp + kk
                                    nc.tensor.matmul(
                                        ctx[par][0:65, :],
                                        v_sb[:, kc, hl, :],
                                        attn[:, kk * 512:(kk + 1) * 512],
                                        start=(kc == 0),
                                        stop=(kc == kcmax - 1),
                                    )
                        # normalize both heads of the pair into ct_qg.
                        # One batched reciprocal; denominator rows live at
                        # partitions 0 and 32 (the only legal operand bases).
                        den = smalls.tile([33, 512], F32, tag="den")
                        nc.gpsimd.memset(den[:], 1.0)
                        for par in range(2):
                            nc.vector.tensor_copy(
                                den[32 * par:32 * par + 1, :],
                                ctx[par][64:65, :],
                            )
                        rec = smalls.tile([33, 512], MMD, tag="rec")
                        with nc.allow_low_precision(reason="f32r==fp32"):
                            nc.vector.reciprocal(rec[:], den[:])
                        recb = smalls.tile([1, 512], MMD, tag="recb")
                        nc.vector.tensor_copy(recb[:], rec[32:33, :])
                        rec_rows = (rec[0:1, :], recb[:])
                        for par in range(2):
                            bc_ps = bcps.tile([128, 512], F32, tag="bc")
                            nc.tensor.matmul(
                                bc_ps[0:64, :], ones_r[0:1, 0:64],
                                rec_rows[par],
                            )
                            bc_sb = smalls.tile([64, 512], F32, tag="bcs")
                            nc.vector.tensor_copy(bc_sb[:], bc_ps[0:64, :])
                            bp = par * 64
                            nc.vector.tensor_tensor(
                                ct_qg[bp:bp + 64, pj, :],
                                ctx[par][0:64, :], bc_sb[:], ALU.mult,
                            )

                    # ---- output projection for this q-group ----
                    for ns in range(4):
                        nt_i = qg * 4 + ns
                        nsl = slice(ns * 128, (ns + 1) * 128)
                        for oh in range(2):
                            po = bcps.tile([128, 512], F32, tag="bc")
                            for j in range(4):
                                nc.tensor.matmul(
                                    po[:],
                                    ct_qg[:, j, nsl],
                                    woT_sb[:, j, oh * 512:(oh + 1) * 512],
                                    start=(j == 0), stop=(j == 3),
                                )
                            ot = ostage.tile([128, 512], F32, tag="ot")
                            nc.vector.tensor_copy(ot[:], po[:])
                            nc.sync.dma_start(
                                pout_d[nt_i * 128:(nt_i + 1) * 128,
                                       oh * 512:(oh + 1) * 512],
                                ot[:],
                            )
    nc.compile()
    return nc


def _host_prepare(inputs):
    """Split the full problem into 8 per-core input maps + host-side info."""
    q = np.asarray(inputs["query"], dtype=np.float32)
    k = np.asarray(inputs["key"], dtype=np.float32)
    v = np.asarray(inputs["value"], dtype=np.float32)
    mask = np.asarray(inputs["mask"])
    w = {n: np.asarray(inputs[n], dtype=np.float32)
         for n in ("wq1", "wq2", "wk1", "wk2", "wv1", "wv2", "wo")}
    bias = {n: np.asarray(inputs[n], dtype=np.float32)
            for n in ("bq1", "bq2", "bk1", "bk2", "bv1", "bv2", "bo")}

    m = mask.reshape(S, S)
    if np.array_equal(m != 0, np.tril(np.ones((S, S), bool))):
        mask_mode = "causal"
    elif np.all(m != 0):
        mask_mode = "full"
    else:
        mask_mode = "general"

    pat = None
    m01T = None
    if mask_mode == "causal":
        kk = np.arange(128)[:, None]
        qq = np.arange(512)[None, :]
        pat = np.stack(
            [(kk + 128 * i <= qq).astype(np.float32) for i in range(4)], axis=1
        )  # [128, 4, 512]
        pat = np.ascontiguousarray(pat)
    elif mask_mode == "general":
        m01T = np.ascontiguousarray((m != 0).T.astype(np.float32))

    scale = 1.0 / np.sqrt(DK).astype(np.float32)

    if MM_DTYPE == "bf16":
        import ml_dtypes

        mmd_np = ml_dtypes.bfloat16
    else:
        mmd_np = np.float32

    def cvt(a):
        return np.ascontiguousarray(a).astype(mmd_np)

    in_maps = []
    for c in range(NCORES):
        b, g = divmod(c, 2)
        sl = slice(g * GCH, (g + 1) * GCH)
        im = {
            "xqT": cvt(q[b].T),
            "xkT": cvt(k[b].T),
            "xvT": cvt(v[b].T),
            "w1T_q": cvt(w["wq1"][sl].T),
            # fold the 1/sqrt(dk) score scale into the non-silu Q branch,
            # and 0.5 everywhere (silu computed as A*(1+tanh(A/2)) = 2*silu)
            "w2T_q": cvt(w["wq2"][sl].T * (scale * 0.5)),
            "w2T_k": cvt(w["wk2"][sl].T * 0.5),
            "w2T_v": cvt(w["wv2"][sl].T * 0.5),
            "w1T_k": cvt(w["wk1"][sl].T),
            "w1T_v": cvt(w["wv1"][sl].T),
            "b1_q": np.ascontiguousarray(bias["bq1"][sl].reshape(4, 128).T),
            "b1h_q": np.ascontiguousarray(
                (bias["bq1"][sl] * 0.5).reshape(4, 128).T),
            "b2_q": np.ascontiguousarray(
                (bias["bq2"][sl] * (scale * 0.5)).reshape(4, 128).T),
            "b1_k": np.ascontiguousarray(bias["bk1"][sl].reshape(4, 128).T),
            "b1h_k": np.ascontiguousarray(
                (bias["bk1"][sl] * 0.5).reshape(4, 128).T),
            "b2_k": np.ascontiguousarray(
                (bias["bk2"][sl] * 0.5).reshape(4, 128).T),
            "b1_v": cvt(bias["bv1"][sl].reshape(1, GCH)),
            "b2_v": cvt((bias["bv2"][sl] * 0.5).reshape(1, GCH)),
            "woT": cvt(
                w["wo"][:, sl].T.reshape(4, 128, D).transpose(1, 0, 2)),
        }
        if mask_mode == "causal":
            im["pat"] = cvt(pat)
        elif mask_mode == "general":
            im["m01T"] = cvt(m01T)
        in_maps.append(im)
    return mask_mode, in_maps, bias["bo"]


def kernel(**inputs):
    global LAST_RESULT
    mask_mode, in_maps, bo = _host_prepare(inputs)
    nc = build_program(mask_mode)

    import concourse.bass_utils as bu

    if TRACE:
        import types

        try:
            from trn_agent_boot.trn_boot import _ntff_profile_via_ctypes

            hook = _ntff_profile_via_ctypes("/opt/axon/libaxon_pjrt.so")
            m = types.ModuleType("antenv.axon_hooks")
            m.get_axon_ntff_profile_hook = lambda: hook
            import antenv  # noqa: F401

            sys.modules["antenv.axon_hooks"] = m
            bu.upload_artifacts = lambda d: "local://skipped"
        except Exception as e:
            print("profiling hook install failed:", e)

    res = bu.run_bass_kernel_spmd(
        nc, in_maps, core_ids=list(range(NCORES)),
        trace=TRACE, trace_cores=TRACE_CORES,
    )
    LAST_RESULT = res

    out = np.empty((B, S, D), dtype=np.float32)
    for b in range(B):
        out[b] = (res.results[2 * b]["pout"] + res.results[2 * b + 1]["pout"]
                  + bo[None, :])
    return out


# revision 22
# speedup vs baseline: 1.2913x; 1.0263x over previous
"""SwiGLU-projected causal MHA (B=4, S=2048, D=1024, H=16) on 8 TRN2 NeuronCores.

Baseline (572888 ns) restored from the original staged kernel.

Sharding: core c -> (batch b = c//2, head-group g = c%2).  Each core computes
the SwiGLU Q/K/V projections for its 512 output channels (= 8 heads) of its
batch, runs causal attention for those heads, and produces a partial output
projection (contraction over its 512 channels).  The host sums the two
partials per batch and adds the output bias.
"""
import sys

sys.path.insert(0, "/opt/trn_rl_repo")
import numpy as np

import concourse.bacc as bacc
import concourse.tile as tile
import concourse.mybir as mybir

B, S, D = 4, 2048, 1024
H, DK = 16, 64
NCORES = 8
GCH = 512          # channels per core (8 heads)
NT = S // 128      # 16 seq chunks
F32 = mybir.dt.float32
F32R = mybir.dt.float32r
ACTF = mybir.ActivationFunctionType
ALU = mybir.AluOpType

TRACE = False          # set by test.py for profiling runs
TRACE_CORES = None
LAST_RESULT = None     # BassKernelResults stash for test.py
MM_DTYPE = "bf16"      # "bf16" (fast weight load) or "f32r" (higher precision)


def build_program(mask_mode):
    """mask_mode: 'causal' (tril), 'full' (all ones), 'general' (arbitrary)."""
    MMD = mybir.dt.bfloat16 if MM_DTYPE == "bf16" else F32R
    nc = bacc.Bacc("TRN2", target_bir_lowering=False, debug=False)

    xT = {s: nc.dram_tensor(f"x{s}T", [D, S], MMD, kind="ExternalInput")
          for s in "qkv"}
    w1T = {s: nc.dram_tensor(f"w1T_{s}", [D, GCH], MMD, kind="ExternalInput")
           for s in "qkv"}
    w2T = {s: nc.dram_tensor(f"w2T_{s}", [D, GCH], MMD, kind="ExternalInput")
           for s in "qkv"}
    bias_d = {}
    for s in "qk":
        for bn in ("b1", "b2", "b1h"):
            bias_d[f"{bn}_{s}"] = nc.dram_tensor(f"{bn}_{s}", [128, 4], F32,
                                                 kind="ExternalInput")
    b1v_d = nc.dram_tensor("b1_v", [1, GCH], MMD, kind="ExternalInput")
    b2v_d = nc.dram_tensor("b2_v", [1, GCH], MMD, kind="ExternalInput")
    woT_d = nc.dram_tensor("woT", [128, 4, D], MMD, kind="ExternalInput")
    pat_d = m01T_d = None
    if mask_mode == "causal":
        pat_d = nc.dram_tensor("pat", [128, 4, 512], MMD, kind="ExternalInput")
    elif mask_mode == "general":
        m01T_d = nc.dram_tensor("m01T", [S, S], MMD, kind="ExternalInput")
    pout_d = nc.dram_tensor("pout", [S, D], F32, kind="ExternalOutput")

    def kc_count(qg):
        return 4 * qg + 4 if mask_mode == "causal" else NT

    with tile.TileContext(nc) as tc:
        with (
            tc.tile_pool(name="persist", bufs=1) as persist,
        ):
            qt_sb = persist.tile([128, 4, S], MMD, tag="qt")
            kt_sb = persist.tile([128, 4, S], MMD, tag="kt")
            v_sb = persist.tile([128, NT, 8, 65], MMD, tag="v")
            woT_sb = persist.tile([128, 4, D], MMD, tag="wo")
            nc.sync.dma_start(woT_sb[:], woT_d[:])
            onesf = persist.tile([1, 128], F32, tag="onesf")
            ones_r = persist.tile([1, 128], MMD, tag="ones_r")
            nc.any.memset(onesf[:], 1.0)
            nc.vector.tensor_copy(ones_r[:], onesf[:])
            onescol = persist.tile([128, 1], F32, tag="onescol")
            nc.any.memset(onescol[:], 1.0)
            nc.vector.tensor_copy(
                v_sb[:, :, :, 64:65],
                onescol[:, None, :].to_broadcast([128, NT, 8, 1]),
            )
            if mask_mode == "causal":
                pat_sb = persist.tile([128, 4, 512], MMD, tag="pat")
                nc.sync.dma_start(pat_sb[:], pat_d[:])

            # ---------------- Phase A: SwiGLU projections ----------------
            with (
                tc.tile_pool(name="wpool", bufs=4) as wpool,
                tc.tile_pool(name="xpool", bufs=12) as xpool,
                tc.tile_pool(name="stage", bufs=4) as stage,
                tc.tile_pool(name="pps", bufs=6, space="PSUM") as pps,
            ):
                for s in "vkq":
                    w1sb = wpool.tile([128, 8, GCH], MMD, tag="w")
                    w2sb = wpool.tile([128, 8, GCH], MMD, tag="w")
                    nc.sync.dma_start(
                        w1sb[:], w1T[s][:].rearrange("(dc p) o -> p dc o", p=128)
                    )
                    nc.sync.dma_start(
                        w2sb[:], w2T[s][:].rearrange("(dc p) o -> p dc o", p=128)
                    )
                    if s != "v":
                        b1sb = persist.tile([128, 4], F32, tag=f"b1{s}")
                        b2sb = persist.tile([128, 4], F32, tag=f"b2{s}")
                        b1hsb = persist.tile([128, 4], F32, tag=f"b1h{s}")
                        nc.sync.dma_start(b1sb[:], bias_d[f"b1_{s}"][:])
                        nc.sync.dma_start(b2sb[:], bias_d[f"b2_{s}"][:])
                        nc.sync.dma_start(b1hsb[:], bias_d[f"b1h_{s}"][:])
                    else:
                        b1vr = persist.tile([1, GCH], MMD, tag="b1v")
                        b2vr = persist.tile([1, GCH], MMD, tag="b2v")
                        nc.sync.dma_start(b1vr[:], b1v_d[:])
                        nc.sync.dma_start(b2vr[:], b2v_d[:])

                    for t in range(4):  # 512-wide seq tiles
                        xts = []
                        for dc in range(8):
                            xt = xpool.tile([128, 512], MMD, tag="xt")
                            nc.sync.dma_start(
                                xt[:],
                                xT[s][dc * 128:(dc + 1) * 128,
                                      t * 512:(t + 1) * 512],
                            )
                            xts.append(xt)
                        for jh in range(2):
                            ps1 = [pps.tile([128, 512], F32, tag="pp",
                                            name=f"ps1_{i}")
                                   for i in range(2)]
                            ps2 = [pps.tile([128, 512], F32, tag="pp",
                                            name=f"ps2_{i}")
                                   for i in range(2)]
                            for dc in range(8):
                                for jj in range(2):
                                    j = jh * 2 + jj
                                    if s == "v":
                                        # seq on partitions: lhsT = x chunk
                                        nc.tensor.matmul(
                                            ps1[jj][:],
                                            xts[dc][:, j * 128:(j + 1) * 128],
                                            w1sb[:, dc, :],
                                            start=(dc == 0), stop=False,
                                        )
                                        nc.tensor.matmul(
                                            ps2[jj][:],
                                            xts[dc][:, j * 128:(j + 1) * 128],
                                            w2sb[:, dc, :],
                                            start=(dc == 0), stop=False,
                                        )
                                    else:
                                        # channels on partitions: lhsT = w chunk
                                        nc.tensor.matmul(
                                            ps1[jj][:],
                                            w1sb[:, dc, j * 128:(j + 1) * 128],
                                            xts[dc][:],
                                            start=(dc == 0), stop=(dc == 7),
                                        )
                                        nc.tensor.matmul(
                                            ps2[jj][:],
                                            w2sb[:, dc, j * 128:(j + 1) * 128],
                                            xts[dc][:],
                                            start=(dc == 0), stop=(dc == 7),
                                        )
                            for jj in range(2):
                                j = jh * 2 + jj
                                act = stage.tile([128, 512], F32, tag="act")
                                if s == "v":
                                    # fold the biases into the accumulation
                                    # (they vary along the free/channel dim)
                                    nc.tensor.matmul(
                                        ps1[jj][:], ones_r[:], b1vr[:],
                                        start=False, stop=True,
                                    )
                                    nc.tensor.matmul(
                                        ps2[jj][:], ones_r[:], b2vr[:],
                                        start=False, stop=True,
                                    )
                                    nc.scalar.activation(
                                        act[:], ps1[jj][:], ACTF.Tanh,
                                        scale=0.5,
                                    )
                                    u = stage.tile([128, 512], F32, tag="u")
                                    nc.vector.tensor_tensor(
                                        u[:], ps1[jj][:], act[:], ALU.mult
                                    )
                                    nc.vector.tensor_tensor(
                                        act[:], ps1[jj][:], u[:], ALU.add
                                    )
                                    nt_i = t * 4 + j
                                    nc.vector.tensor_tensor(
                                        v_sb[:, nt_i, :, 0:64],
                                        ps2[jj][:].rearrange(
                                            "p (h d) -> p h d", h=8
                                        ),
                                        act[:].rearrange(
                                            "p (h d) -> p h d", h=8
                                        ),
                                        ALU.mult,
                                    )
                                else:
                                    bias1 = b1sb[:, j:j + 1]
                                    bias2 = b2sb[:, j:j + 1]
                                    # act = tanh((A)/2), A = ps1 + b1
                                    nc.scalar.activation(
                                        act[:], ps1[jj][:], ACTF.Tanh,
                                        scale=0.5, bias=b1hsb[:, j:j + 1],
                                    )
                                    a_sb = stage.tile([128, 512], F32,
                                                      tag="u")
                                    nc.vector.tensor_scalar_add(
                                        a_sb[:], ps1[jj][:], bias1
                                    )
                                    # act = A*(1+tanh(A/2)) = 2*silu(A)
                                    nc.vector.scalar_tensor_tensor(
                                        act[:], act[:], 1.0, a_sb[:],
                                        op0=ALU.add, op1=ALU.mult,
                                    )
                                    dst = (qt_sb if s == "q" else kt_sb)[
                                        :, j, t * 512:(t + 1) * 512
                                    ]
                                    nc.vector.scalar_tensor_tensor(
                                        dst, ps2[jj][:], bias2, act[:],
                                        op0=ALU.add, op1=ALU.mult,
                                    )

            # ------------- Phase B+C: attention + output projection -------
            with (
                tc.tile_pool(name="scps", bufs=2, space="PSUM") as scps,
                tc.tile_pool(name="cxps", bufs=3, space="PSUM") as cxps,
                tc.tile_pool(name="bcps", bufs=1, space="PSUM") as bcps,
                tc.tile_pool(name="apool", bufs=6) as apool,
                tc.tile_pool(name="ctpool", bufs=2) as ctpool,
                tc.tile_pool(name="smalls", bufs=4) as smalls,
                tc.tile_pool(name="ostage", bufs=4) as ostage,
                tc.tile_pool(name="mpool", bufs=2) as mpool,
            ):
                for qg in range(4):
                    kcmax = kc_count(qg)
                    qsl = slice(qg * 512, (qg + 1) * 512)
                    ct_qg = ctpool.tile([128, 4, 512], MMD, tag="ct")

                    mtiles = None
                    if mask_mode == "general":
                        mtiles = []
                        mt_sb = mpool.tile([128, NT, 512], MMD, tag="mt")
                        for kc in range(kcmax):
                            nc.sync.dma_start(
                                mt_sb[:, kc, :],
                                m01T_d[kc * 128:(kc + 1) * 128, qsl],
                            )
                            mtiles.append(mt_sb[:, kc, :])

                    for pj in range(4):   # head pair: hl = 2*pj (+1)
                        ctx = [cxps.tile([128, 512], F32, tag="cx",
                                         name=f"ctx_{i}")
                               for i in range(2)]
                        for kc in range(kcmax):
                            ksl = slice(kc * 128, (kc + 1) * 128)
                            sc2 = scps.tile([128, 1024], F32, tag="sc",
                                            name="sc2")
                            for par in range(2):
                                bp = par * 64
                                nc.tensor.matmul(
                                    sc2[:, par * 512:(par + 1) * 512],
                                    kt_sb[bp:bp + 64, pj, ksl],
                                    qt_sb[bp:bp + 64, pj, qsl],
                                )
                            attn = apool.tile([128, 1024], MMD, tag="at")
                            nc.scalar.activation(attn[:], sc2[:], ACTF.Exp)
                            if mask_mode == "causal" and kc >= 4 * qg:
                                nc.vector.tensor_tensor(
                                    attn[:].rearrange("p (a b) -> p a b", a=2),
                                    attn[:].rearrange("p (a b) -> p a b", a=2),
                                    pat_sb[:, kc - 4 * qg, :][:, None, :]
                                    .to_broadcast([128, 2, 512]),
                                    ALU.mult,
                                )
                            elif mask_mode == "general":
                                for par in range(2):
                                    asl = attn[:, par * 512:(par + 1) * 512]
                                    nc.vector.tensor_tensor(
                                        asl, asl, mtiles[kc], ALU.mult,
                                    )
                            for par in range(2):
                                hl = 2 * pj + par
                                nc.tensor.matmul(
                                    ctx[par][0:65, :],
                                    v_sb[:, kc, hl, :],
                                    attn[:, par * 512:(par + 1) * 512],
                                    start=(kc == 0),
                                    stop=(kc == kcmax - 1),
                                )
                        # normalize both heads of the pair into ct_qg.
                        # One batched reciprocal; denominator rows live at
                        # partitions 0 and 32 (the only legal operand bases).
                        den = smalls.tile([33, 512], F32, tag="den")
                        nc.gpsimd.memset(den[:], 1.0)
                        for par in range(2):
                            nc.vector.tensor_copy(
                                den[32 * par:32 * par + 1, :],
                                ctx[par][64:65, :],
                            )
                        rec = smalls.tile([33, 512], MMD, tag="rec")
                        with nc.allow_low_precision(reason="f32r==fp32"):
                            nc.vector.reciprocal(rec[:], den[:])
                        recb = smalls.tile([1, 512], MMD, tag="recb")
                        nc.vector.tensor_copy(recb[:], rec[32:33, :])
                        rec_rows = (rec[0:1, :], recb[:])
                        for par in range(2):
                            bc_ps = bcps.tile([128, 512], F32, tag="bc")
                            nc.tensor.matmul(
                                bc_ps[0:64, :], ones_r[0:1, 0:64],
                                rec_rows[par],
                            )
                            bc_sb = smalls.tile([64, 512], F32, tag="bcs")
                            nc.vector.tensor_copy(bc_sb[:], bc_ps[0:64, :])
                            bp = par * 64
                            nc.vector.tensor_tensor(
                                ct_qg[bp:bp + 64, pj, :],
                                ctx[par][0:64, :], bc_sb[:], ALU.mult,
                            )

                    # ---- output projection for this q-group ----
                    for ns in range(4):
                        nt_i = qg * 4 + ns
                        nsl = slice(ns * 128, (ns + 1) * 128)
                        for oh in range(2):
                            po = bcps.tile([128, 512], F32, tag="bc")
                            for j in range(4):
                                nc.tensor.matmul(
                                    po[:],
                                    ct_qg[:, j, nsl],
                                    woT_sb[:, j, oh * 512:(oh + 1) * 512],
                                    start=(j == 0), stop=(j == 3),
                                )
                            ot = ostage.tile([128, 512], F32, tag="ot")
                            nc.vector.tensor_copy(ot[:], po[:])
                            nc.sync.dma_start(
                                pout_d[nt_i * 128:(nt_i + 1) * 128,
                                       oh * 512:(oh + 1) * 512],
                                ot[:],
                            )
    nc.compile()
    return nc


def _host_prepare(inputs):
    """Split the full problem into 8 per-core input maps + host-side info."""
    q = np.asarray(inputs["query"], dtype=np.float32)
    k = np.asarray(inputs["key"], dtype=np.float32)
    v = np.asarray(inputs["value"], dtype=np.float32)
    mask = np.asarray(inputs["mask"])
    w = {n: np.asarray(inputs[n], dtype=np.float32)
         for n in ("wq1", "wq2", "wk1", "wk2", "wv1", "wv2", "wo")}
    bias = {n: np.asarray(inputs[n], dtype=np.float32)
            for n in ("bq1", "bq2", "bk1", "bk2", "bv1", "bv2", "bo")}

    m = mask.reshape(S, S)
    if np.array_equal(m != 0, np.tril(np.ones((S, S), bool))):
        mask_mode = "causal"
    elif np.all(m != 0):
        mask_mode = "full"
    else:
        mask_mode = "general"

    pat = None
    m01T = None
    if mask_mode == "causal":
        kk = np.arange(128)[:, None]
        qq = np.arange(512)[None, :]
        pat = np.stack(
            [(kk + 128 * i <= qq).astype(np.float32) for i in range(4)], axis=1
        )  # [128, 4, 512]
        pat = np.ascontiguousarray(pat)
    elif mask_mode == "general":
        m01T = np.ascontiguousarray((m != 0).T.astype(np.float32))

    scale = 1.0 / np.sqrt(DK).astype(np.float32)

    if MM_DTYPE == "bf16":
        import ml_dtypes

        mmd_np = ml_dtypes.bfloat16
    else:
        mmd_np = np.float32

    def cvt(a):
        return np.ascontiguousarray(a).astype(mmd_np)

    in_maps = []
    for c in range(NCORES):
        b, g = divmod(c, 2)
        sl = slice(g * GCH, (g + 1) * GCH)
        im = {
            "xqT": cvt(q[b].T),
            "xkT": cvt(k[b].T),
            "xvT": cvt(v[b].T),
            "w1T_q": cvt(w["wq1"][sl].T),
            # fold the 1/sqrt(dk) score scale into the non-silu Q branch,
            # and 0.5 everywhere (silu computed as A*(1+tanh(A/2)) = 2*silu)
            "w2T_q": cvt(w["wq2"][sl].T * (scale * 0.5)),
            "w2T_k": cvt(w["wk2"][sl].T * 0.5),
            "w2T_v": cvt(w["wv2"][sl].T * 0.5),
            "w1T_k": cvt(w["wk1"][sl].T),
            "w1T_v": cvt(w["wv1"][sl].T),
            "b1_q": np.ascontiguousarray(bias["bq1"][sl].reshape(4, 128).T),
            "b1h_q": np.ascontiguousarray(
                (bias["bq1"][sl] * 0.5).reshape(4, 128).T),
            "b2_q": np.ascontiguousarray(
                (bias["bq2"][sl] * (scale * 0.5)).reshape(4, 128).T),
            "b1_k": np.ascontiguousarray(bias["bk1"][sl].reshape(4, 128).T),
            "b1h_k": np.ascontiguousarray(
                (bias["bk1"][sl] * 0.5).reshape(4, 128).T),
            "b2_k": np.ascontiguousarray(
                (bias["bk2"][sl] * 0.5).reshape(4, 128).T),
            "b1_v": cvt(bias["bv1"][sl].reshape(1, GCH)),
            "b2_v": cvt((bias["bv2"][sl] * 0.5).reshape(1, GCH)),
            "woT": cvt(
                w["wo"][:, sl].T.reshape(4, 128, D).transpose(1, 0, 2)),
        }
        if mask_mode == "causal":
            im["pat"] = cvt(pat)
        elif mask_mode == "general":
            im["m01T"] = cvt(m01T)
        in_maps.append(im)
    return mask_mode, in_maps, bias["bo"]


def kernel(**inputs):
    global LAST_RESULT
    mask_mode, in_maps, bo = _host_prepare(inputs)
    nc = build_program(mask_mode)

    import concourse.bass_utils as bu

    if TRACE:
        import types

        try:
            from trn_agent_boot.trn_boot import _ntff_profile_via_ctypes

            hook = _ntff_profile_via_ctypes("/opt/axon/libaxon_pjrt.so")
            m = types.ModuleType("antenv.axon_hooks")
            m.get_axon_ntff_profile_hook = lambda: hook
            import antenv  # noqa: F401

            sys.modules["antenv.axon_hooks"] = m
            bu.upload_artifacts = lambda d: "local://skipped"
        except Exception as e:
            print("profiling hook install failed:", e)

    res = bu.run_bass_kernel_spmd(
        nc, in_maps, core_ids=list(range(NCORES)),
        trace=TRACE, trace_cores=TRACE_CORES,
    )
    LAST_RESULT = res

    out = np.empty((B, S, D), dtype=np.float32)
    for b in range(B):
        out[b] = (res.results[2 * b]["pout"] + res.results[2 * b + 1]["pout"]
                  + bo[None, :])
    return out


# revision 23
# speedup vs baseline: 1.3224x; 1.0240x over previous
"""SwiGLU-projected causal MHA (B=4, S=2048, D=1024, H=16) on 8 TRN2 NeuronCores.

Baseline (572888 ns) restored from the original staged kernel.

Sharding: core c -> (batch b = c//2, head-group g = c%2).  Each core computes
the SwiGLU Q/K/V projections for its 512 output channels (= 8 heads) of its
batch, runs causal attention for those heads, and produces a partial output
projection (contraction over its 512 channels).  The host sums the two
partials per batch and adds the output bias.
"""
import sys

sys.path.insert(0, "/opt/trn_rl_repo")
import numpy as np

import concourse.bacc as bacc
import concourse.tile as tile
import concourse.mybir as mybir

B, S, D = 4, 2048, 1024
H, DK = 16, 64
NCORES = 8
GCH = 512          # channels per core (8 heads)
NT = S // 128      # 16 seq chunks
F32 = mybir.dt.float32
F32R = mybir.dt.float32r
ACTF = mybir.ActivationFunctionType
ALU = mybir.AluOpType

TRACE = False          # set by test.py for profiling runs
TRACE_CORES = None
LAST_RESULT = None     # BassKernelResults stash for test.py
MM_DTYPE = "bf16"      # "bf16" (fast weight load) or "f32r" (higher precision)


def build_program(mask_mode):
    """mask_mode: 'causal' (tril), 'full' (all ones), 'general' (arbitrary)."""
    MMD = mybir.dt.bfloat16 if MM_DTYPE == "bf16" else F32R
    nc = bacc.Bacc("TRN2", target_bir_lowering=False, debug=False)

    xT = {s: nc.dram_tensor(f"x{s}T", [D, S], MMD, kind="ExternalInput")
          for s in "qkv"}
    w1T = {s: nc.dram_tensor(f"w1T_{s}", [D, GCH], MMD, kind="ExternalInput")
           for s in "qkv"}
    w2T = {s: nc.dram_tensor(f"w2T_{s}", [D, GCH], MMD, kind="ExternalInput")
           for s in "qkv"}
    bias_d = {}
    for s in "qk":
        for bn in ("b1", "b2", "b1h"):
            bias_d[f"{bn}_{s}"] = nc.dram_tensor(f"{bn}_{s}", [128, 4], F32,
                                                 kind="ExternalInput")
    b1v_d = nc.dram_tensor("b1_v", [1, GCH], MMD, kind="ExternalInput")
    b2v_d = nc.dram_tensor("b2_v", [1, GCH], MMD, kind="ExternalInput")
    woT_d = nc.dram_tensor("woT", [128, 4, D], MMD, kind="ExternalInput")
    pat_d = m01T_d = None
    if mask_mode == "causal":
        pat_d = nc.dram_tensor("pat", [128, 4, 512], MMD, kind="ExternalInput")
    elif mask_mode == "general":
        m01T_d = nc.dram_tensor("m01T", [S, S], MMD, kind="ExternalInput")
    pout_d = nc.dram_tensor("pout", [S, D], F32, kind="ExternalOutput")

    def kc_count(qg):
        return 4 * qg + 4 if mask_mode == "causal" else NT

    with tile.TileContext(nc) as tc:
        with (
            tc.tile_pool(name="persist", bufs=1) as persist,
        ):
            qt_sb = persist.tile([128, 4, S], MMD, tag="qt")
            kt_sb = persist.tile([128, 4, S], MMD, tag="kt")
            v_sb = persist.tile([128, NT, 8, 65], MMD, tag="v")
            woT_sb = persist.tile([128, 4, D], MMD, tag="wo")
            nc.sync.dma_start(woT_sb[:], woT_d[:])
            onesf = persist.tile([1, 128], F32, tag="onesf")
            ones_r = persist.tile([1, 128], MMD, tag="ones_r")
            nc.any.memset(onesf[:], 1.0)
            nc.vector.tensor_copy(ones_r[:], onesf[:])
            onescol = persist.tile([128, 1], F32, tag="onescol")
            nc.any.memset(onescol[:], 1.0)
            nc.vector.tensor_copy(
                v_sb[:, :, :, 64:65],
                onescol[:, None, :].to_broadcast([128, NT, 8, 1]),
            )
            if mask_mode == "causal":
                pat_sb = persist.tile([128, 4, 512], MMD, tag="pat")
                nc.sync.dma_start(pat_sb[:], pat_d[:])

            # ---------------- Phase A: SwiGLU projections ----------------
            with (
                tc.tile_pool(name="wpool", bufs=4) as wpool,
                tc.tile_pool(name="xpool", bufs=12) as xpool,
                tc.tile_pool(name="stage", bufs=4) as stage,
                tc.tile_pool(name="pps", bufs=6, space="PSUM") as pps,
            ):
                for s in "vkq":
                    w1sb = wpool.tile([128, 8, GCH], MMD, tag="w")
                    w2sb = wpool.tile([128, 8, GCH], MMD, tag="w")
                    nc.sync.dma_start(
                        w1sb[:], w1T[s][:].rearrange("(dc p) o -> p dc o", p=128)
                    )
                    nc.sync.dma_start(
                        w2sb[:], w2T[s][:].rearrange("(dc p) o -> p dc o", p=128)
                    )
                    if s != "v":
                        b1sb = persist.tile([128, 4], F32, tag=f"b1{s}")
                        b2sb = persist.tile([128, 4], F32, tag=f"b2{s}")
                        b1hsb = persist.tile([128, 4], F32, tag=f"b1h{s}")
                        nc.sync.dma_start(b1sb[:], bias_d[f"b1_{s}"][:])
                        nc.sync.dma_start(b2sb[:], bias_d[f"b2_{s}"][:])
                        nc.sync.dma_start(b1hsb[:], bias_d[f"b1h_{s}"][:])
                    else:
                        b1vr = persist.tile([1, GCH], MMD, tag="b1v")
                        b2vr = persist.tile([1, GCH], MMD, tag="b2v")
                        nc.sync.dma_start(b1vr[:], b1v_d[:])
                        nc.sync.dma_start(b2vr[:], b2v_d[:])

                    for t in range(4):  # 512-wide seq tiles
                        xts = []
                        for dc in range(8):
                            xt = xpool.tile([128, 512], MMD, tag="xt")
                            nc.sync.dma_start(
                                xt[:],
                                xT[s][dc * 128:(dc + 1) * 128,
                                      t * 512:(t + 1) * 512],
                            )
                            xts.append(xt)
                        for jh in range(2):
                            ps1 = [pps.tile([128, 512], F32, tag="pp",
                                            name=f"ps1_{i}")
                                   for i in range(2)]
                            ps2 = [pps.tile([128, 512], F32, tag="pp",
                                            name=f"ps2_{i}")
                                   for i in range(2)]
                            for dc in range(8):
                                for jj in range(2):
                                    j = jh * 2 + jj
                                    if s == "v":
                                        # seq on partitions: lhsT = x chunk
                                        nc.tensor.matmul(
                                            ps1[jj][:],
                                            xts[dc][:, j * 128:(j + 1) * 128],
                                            w1sb[:, dc, :],
                                            start=(dc == 0), stop=False,
                                        )
                                        nc.tensor.matmul(
                                            ps2[jj][:],
                                            xts[dc][:, j * 128:(j + 1) * 128],
                                            w2sb[:, dc, :],
                                            start=(dc == 0), stop=False,
                                        )
                                    else:
                                        # channels on partitions: lhsT = w chunk
                                        nc.tensor.matmul(
                                            ps1[jj][:],
                                            w1sb[:, dc, j * 128:(j + 1) * 128],
                                            xts[dc][:],
                                            start=(dc == 0), stop=(dc == 7),
                                        )
                                        nc.tensor.matmul(
                                            ps2[jj][:],
                                            w2sb[:, dc, j * 128:(j + 1) * 128],
                                            xts[dc][:],
                                            start=(dc == 0), stop=(dc == 7),
                                        )
                            for jj in range(2):
                                j = jh * 2 + jj
                                act = stage.tile([128, 512], F32, tag="act")
                                if s == "v":
                                    # fold the biases into the accumulation
                                    # (they vary along the free/channel dim)
                                    nc.tensor.matmul(
                                        ps1[jj][:], ones_r[:], b1vr[:],
                                        start=False, stop=True,
                                    )
                                    nc.tensor.matmul(
                                        ps2[jj][:], ones_r[:], b2vr[:],
                                        start=False, stop=True,
                                    )
                                    nc.scalar.activation(
                                        act[:], ps1[jj][:], ACTF.Tanh,
                                        scale=0.5,
                                    )
                                    u = stage.tile([128, 512], F32, tag="u")
                                    nc.vector.tensor_tensor(
                                        u[:], ps1[jj][:], act[:], ALU.mult
                                    )
                                    nc.vector.tensor_tensor(
                                        act[:], ps1[jj][:], u[:], ALU.add
                                    )
                                    nt_i = t * 4 + j
                                    nc.vector.tensor_tensor(
                                        v_sb[:, nt_i, :, 0:64],
                                        ps2[jj][:].rearrange(
                                            "p (h d) -> p h d", h=8
                                        ),
                                        act[:].rearrange(
                                            "p (h d) -> p h d", h=8
                                        ),
                                        ALU.mult,
                                    )
                                else:
                                    bias1 = b1sb[:, j:j + 1]
                                    bias2 = b2sb[:, j:j + 1]
                                    # act = tanh((A)/2), A = ps1 + b1
                                    nc.scalar.activation(
                                        act[:], ps1[jj][:], ACTF.Tanh,
                                        scale=0.5, bias=b1hsb[:, j:j + 1],
                                    )
                                    a_sb = stage.tile([128, 512], F32,
                                                      tag="u")
                                    nc.vector.tensor_scalar_add(
                                        a_sb[:], ps1[jj][:], bias1
                                    )
                                    # act = A*(1+tanh(A/2)) = 2*silu(A)
                                    nc.vector.scalar_tensor_tensor(
                                        act[:], act[:], 1.0, a_sb[:],
                                        op0=ALU.add, op1=ALU.mult,
                                    )
                                    dst = (qt_sb if s == "q" else kt_sb)[
                                        :, j, t * 512:(t + 1) * 512
                                    ]
                                    nc.vector.scalar_tensor_tensor(
                                        dst, ps2[jj][:], bias2, act[:],
                                        op0=ALU.add, op1=ALU.mult,
                                    )

            # ------------- Phase B+C: attention + output projection -------
            with (
                tc.tile_pool(name="scps", bufs=2, space="PSUM") as scps,
                tc.tile_pool(name="cxps", bufs=2, space="PSUM") as cxps,
                tc.tile_pool(name="bcps", bufs=2, space="PSUM") as bcps,
                tc.tile_pool(name="apool", bufs=6) as apool,
                tc.tile_pool(name="ctpool", bufs=2) as ctpool,
                tc.tile_pool(name="smalls", bufs=4) as smalls,
                tc.tile_pool(name="ostage", bufs=4) as ostage,
                tc.tile_pool(name="mpool", bufs=2) as mpool,
            ):
                for qg in range(4):
                    kcmax = kc_count(qg)
                    qsl = slice(qg * 512, (qg + 1) * 512)
                    ct_qg = ctpool.tile([128, 4, 512], MMD, tag="ct")

                    mtiles = None
                    if mask_mode == "general":
                        mtiles = []
                        mt_sb = mpool.tile([128, NT, 512], MMD, tag="mt")
                        for kc in range(kcmax):
                            nc.sync.dma_start(
                                mt_sb[:, kc, :],
                                m01T_d[kc * 128:(kc + 1) * 128, qsl],
                            )
                            mtiles.append(mt_sb[:, kc, :])

                    for pj in range(4):   # head pair: hl = 2*pj (+1)
                        ctx = [cxps.tile([128, 512], F32, tag="cx",
                                         name=f"ctx_{i}")
                               for i in range(2)]
                        for kc in range(kcmax):
                            ksl = slice(kc * 128, (kc + 1) * 128)
                            sc2 = scps.tile([128, 1024], F32, tag="sc",
                                            name="sc2")
                            for par in range(2):
                                bp = par * 64
                                nc.tensor.matmul(
                                    sc2[:, par * 512:(par + 1) * 512],
                                    kt_sb[bp:bp + 64, pj, ksl],
                                    qt_sb[bp:bp + 64, pj, qsl],
                                )
                            attn = apool.tile([128, 1024], MMD, tag="at")
                            nc.scalar.activation(attn[:], sc2[:], ACTF.Exp)
                            if mask_mode == "causal" and kc >= 4 * qg:
                                nc.vector.tensor_tensor(
                                    attn[:].rearrange("p (a b) -> p a b", a=2),
                                    attn[:].rearrange("p (a b) -> p a b", a=2),
                                    pat_sb[:, kc - 4 * qg, :][:, None, :]
                                    .to_broadcast([128, 2, 512]),
                                    ALU.mult,
                                )
                            elif mask_mode == "general":
                                for par in range(2):
                                    asl = attn[:, par * 512:(par + 1) * 512]
                                    nc.vector.tensor_tensor(
                                        asl, asl, mtiles[kc], ALU.mult,
                                    )
                            for par in range(2):
                                hl = 2 * pj + par
                                nc.tensor.matmul(
                                    ctx[par][0:65, :],
                                    v_sb[:, kc, hl, :],
                                    attn[:, par * 512:(par + 1) * 512],
                                    start=(kc == 0),
                                    stop=(kc == kcmax - 1),
                                )
                        # snapshot ctx (incl. den row 64) to SBUF right
                        # away so the PSUM banks free for the next head pair;
                        # then batched reciprocal + PE broadcast off-PSUM.
                        cts = [smalls.tile([65, 512], F32, tag=f"cts{i}",
                                           name=f"cts{i}") for i in range(2)]
                        for par in range(2):
                            nc.vector.tensor_copy(cts[par][:], ctx[par][0:65, :])
                        den = smalls.tile([33, 512], F32, tag="den")
                        nc.gpsimd.memset(den[:], 1.0)
                        for par in range(2):
                            nc.scalar.activation(
                                den[32 * par:32 * par + 1, :],
                                cts[par][64:65, :], ACTF.Identity,
                            )
                        rec = smalls.tile([33, 512], MMD, tag="rec")
                        with nc.allow_low_precision(reason="f32r==fp32"):
                            nc.vector.reciprocal(rec[:], den[:])
                        recb = smalls.tile([1, 512], MMD, tag="recb")
                        nc.vector.tensor_copy(recb[:], rec[32:33, :])
                        rec_rows = (rec[0:1, :], recb[:])
                        for par in range(2):
                            bc_ps = bcps.tile([128, 512], F32, tag="bc")
                            nc.tensor.matmul(
                                bc_ps[0:64, :], ones_r[0:1, 0:64],
                                rec_rows[par],
                            )
                            bc_sb = smalls.tile([64, 512], F32, tag="bcs")
                            nc.vector.tensor_copy(bc_sb[:], bc_ps[0:64, :])
                            bp = par * 64
                            nc.vector.tensor_tensor(
                                ct_qg[bp:bp + 64, pj, :],
                                cts[par][0:64, :], bc_sb[:], ALU.mult,
                            )

                    # ---- output projection for this q-group ----
                    for ns in range(4):
                        nt_i = qg * 4 + ns
                        nsl = slice(ns * 128, (ns + 1) * 128)
                        for oh in range(2):
                            po = bcps.tile([128, 512], F32, tag="bc")
                            for j in range(4):
                                nc.tensor.matmul(
                                    po[:],
                                    ct_qg[:, j, nsl],
                                    woT_sb[:, j, oh * 512:(oh + 1) * 512],
                                    start=(j == 0), stop=(j == 3),
                                )
                            ot = ostage.tile([128, 512], F32, tag="ot")
                            nc.vector.tensor_copy(ot[:], po[:])
                            nc.sync.dma_start(
                                pout_d[nt_i * 128:(nt_i + 1) * 128,
                                       oh * 512:(oh + 1) * 512],
                                ot[:],
                            )
    nc.compile()
    return nc


def _host_prepare(inputs):
    """Split the full problem into 8 per-core input maps + host-side info."""
    q = np.asarray(inputs["query"], dtype=np.float32)
    k = np.asarray(inputs["key"], dtype=np.float32)
    v = np.asarray(inputs["value"], dtype=np.float32)
    mask = np.asarray(inputs["mask"])
    w = {n: np.asarray(inputs[n], dtype=np.float32)
         for n in ("wq1", "wq2", "wk1", "wk2", "wv1", "wv2", "wo")}
    bias = {n: np.asarray(inputs[n], dtype=np.float32)
            for n in ("bq1", "bq2", "bk1", "bk2", "bv1", "bv2", "bo")}

    m = mask.reshape(S, S)
    if np.array_equal(m != 0, np.tril(np.ones((S, S), bool))):
        mask_mode = "causal"
    elif np.all(m != 0):
        mask_mode = "full"
    else:
        mask_mode = "general"

    pat = None
    m01T = None
    if mask_mode == "causal":
        kk = np.arange(128)[:, None]
        qq = np.arange(512)[None, :]
        pat = np.stack(
            [(kk + 128 * i <= qq).astype(np.float32) for i in range(4)], axis=1
        )  # [128, 4, 512]
        pat = np.ascontiguousarray(pat)
    elif mask_mode == "general":
        m01T = np.ascontiguousarray((m != 0).T.astype(np.float32))

    scale = 1.0 / np.sqrt(DK).astype(np.float32)

    if MM_DTYPE == "bf16":
        import ml_dtypes

        mmd_np = ml_dtypes.bfloat16
    else:
        mmd_np = np.float32

    def cvt(a):
        return np.ascontiguousarray(a).astype(mmd_np)

    in_maps = []
    for c in range(NCORES):
        b, g = divmod(c, 2)
        sl = slice(g * GCH, (g + 1) * GCH)
        im = {
            "xqT": cvt(q[b].T),
            "xkT": cvt(k[b].T),
            "xvT": cvt(v[b].T),
            "w1T_q": cvt(w["wq1"][sl].T),
            # fold the 1/sqrt(dk) score scale into the non-silu Q branch,
            # and 0.5 everywhere (silu computed as A*(1+tanh(A/2)) = 2*silu)
            "w2T_q": cvt(w["wq2"][sl].T * (scale * 0.5)),
            "w2T_k": cvt(w["wk2"][sl].T * 0.5),
            "w2T_v": cvt(w["wv2"][sl].T * 0.5),
            "w1T_k": cvt(w["wk1"][sl].T),
            "w1T_v": cvt(w["wv1"][sl].T),
            "b1_q": np.ascontiguousarray(bias["bq1"][sl].reshape(4, 128).T),
            "b1h_q": np.ascontiguousarray(
                (bias["bq1"][sl] * 0.5).reshape(4, 128).T),
            "b2_q": np.ascontiguousarray(
                (bias["bq2"][sl] * (scale * 0.5)).reshape(4, 128).T),
            "b1_k": np.ascontiguousarray(bias["bk1"][sl].reshape(4, 128).T),
            "b1h_k": np.ascontiguousarray(
                (bias["bk1"][sl] * 0.5).reshape(4, 128).T),
            "b2_k": np.ascontiguousarray(
                (bias["bk2"][sl] * 0.5).reshape(4, 128).T),
            "b1_v": cvt(bias["bv1"][sl].reshape(1, GCH)),
            "b2_v": cvt((bias["bv2"][sl] * 0.5).reshape(1, GCH)),
            "woT": cvt(
                w["wo"][:, sl].T.reshape(4, 128, D).transpose(1, 0, 2)),
        }
        if mask_mode == "causal":
            im["pat"] = cvt(pat)
        elif mask_mode == "general":
            im["m01T"] = cvt(m01T)
        in_maps.append(im)
    return mask_mode, in_maps, bias["bo"]


def kernel(**inputs):
    global LAST_RESULT
    mask_mode, in_maps, bo = _host_prepare(inputs)
    nc = build_program(mask_mode)

    import concourse.bass_utils as bu

    if TRACE:
        import types

        try:
            from trn_agent_boot.trn_boot import _ntff_profile_via_ctypes

            hook = _ntff_profile_via_ctypes("/opt/axon/libaxon_pjrt.so")
            m = types.ModuleType("antenv.axon_hooks")
            m.get_axon_ntff_profile_hook = lambda: hook
            import antenv  # noqa: F401

            sys.modules["antenv.axon_hooks"] = m
            bu.upload_artifacts = lambda d: "local://skipped"
        except Exception as e:
            print("profiling hook install failed:", e)

    res = bu.run_bass_kernel_spmd(
        nc, in_maps, core_ids=list(range(NCORES)),
        trace=TRACE, trace_cores=TRACE_CORES,
    )
    LAST_RESULT = res

    out = np.empty((B, S, D), dtype=np.float32)
    for b in range(B):
        out[b] = (res.results[2 * b]["pout"] + res.results[2 * b + 1]["pout"]
                  + bo[None, :])
    return out


# revision 25
# speedup vs baseline: 1.4449x; 1.0927x over previous
"""SwiGLU-projected causal MHA (B=4, S=2048, D=1024, H=16) on 8 TRN2 NeuronCores.

Baseline (572888 ns) restored from the original staged kernel.

Sharding: core c -> (batch b = c//2, head-group g = c%2).  Each core computes
the SwiGLU Q/K/V projections for its 512 output channels (= 8 heads) of its
batch, runs causal attention for those heads, and produces a partial output
projection (contraction over its 512 channels).  The host sums the two
partials per batch and adds the output bias.
"""
import sys

sys.path.insert(0, "/opt/trn_rl_repo")
import numpy as np

import concourse.bacc as bacc
import concourse.tile as tile
import concourse.mybir as mybir

B, S, D = 4, 2048, 1024
H, DK = 16, 64
NCORES = 8
GCH = 512          # channels per core (8 heads)
NT = S // 128      # 16 seq chunks
F32 = mybir.dt.float32
F32R = mybir.dt.float32r
ACTF = mybir.ActivationFunctionType
ALU = mybir.AluOpType

TRACE = False          # set by test.py for profiling runs
TRACE_CORES = None
LAST_RESULT = None     # BassKernelResults stash for test.py
MM_DTYPE = "bf16"      # "bf16" (fast weight load) or "f32r" (higher precision)


def build_program(mask_mode):
    """mask_mode: 'causal' (tril), 'full' (all ones), 'general' (arbitrary)."""
    MMD = mybir.dt.bfloat16 if MM_DTYPE == "bf16" else F32R
    nc = bacc.Bacc("TRN2", target_bir_lowering=False, debug=False)

    xT = {s: nc.dram_tensor(f"x{s}T", [D, S], MMD, kind="ExternalInput")
          for s in "qkv"}
    w1T = {s: nc.dram_tensor(f"w1T_{s}", [D, GCH], MMD, kind="ExternalInput")
           for s in "qkv"}
    w2T = {s: nc.dram_tensor(f"w2T_{s}", [D, GCH], MMD, kind="ExternalInput")
           for s in "qkv"}
    bias_d = {}
    for s in "qk":
        for bn in ("b1", "b2", "b1h"):
            bias_d[f"{bn}_{s}"] = nc.dram_tensor(f"{bn}_{s}", [128, 4], F32,
                                                 kind="ExternalInput")
    b1v_d = nc.dram_tensor("b1_v", [1, GCH], MMD, kind="ExternalInput")
    b2v_d = nc.dram_tensor("b2_v", [1, GCH], MMD, kind="ExternalInput")
    woT_d = nc.dram_tensor("woT", [128, 4, D], MMD, kind="ExternalInput")
    pat_d = m01T_d = None
    if mask_mode == "causal":
        pat_d = nc.dram_tensor("pat", [128, 4, 512], MMD, kind="ExternalInput")
    elif mask_mode == "general":
        m01T_d = nc.dram_tensor("m01T", [S, S], MMD, kind="ExternalInput")
    pout_d = nc.dram_tensor("pout", [S, D], F32, kind="ExternalOutput")

    def kc_count(qg):
        return 4 * qg + 4 if mask_mode == "causal" else NT

    with tile.TileContext(nc) as tc:
        with (
            tc.tile_pool(name="persist", bufs=1) as persist,
        ):
            qt_sb = persist.tile([128, 4, S], MMD, tag="qt")
            kt_sb = persist.tile([128, 4, S], MMD, tag="kt")
            v_sb = persist.tile([128, NT, 8, 65], MMD, tag="v")
            woT_sb = persist.tile([128, 4, D], MMD, tag="wo")
            nc.sync.dma_start(woT_sb[:], woT_d[:])
            onesf = persist.tile([33, 128], F32, tag="onesf")
            ones_c = persist.tile([33, 128], MMD, tag="ones_c")
            nc.any.memset(onesf[:], 1.0)
            nc.vector.tensor_copy(ones_c[:], onesf[:])
            ones_r = ones_c[0:1, :]
            onescol = persist.tile([128, 1], F32, tag="onescol")
            nc.any.memset(onescol[:], 1.0)
            nc.vector.tensor_copy(
                v_sb[:, :, :, 64:65],
                onescol[:, None, :].to_broadcast([128, NT, 8, 1]),
            )
            if mask_mode == "causal":
                pat_sb = persist.tile([128, 4, 512], MMD, tag="pat")
                nc.sync.dma_start(pat_sb[:], pat_d[:])

            # ---------------- Phase A: SwiGLU projections ----------------
            with (
                tc.tile_pool(name="wpool", bufs=4) as wpool,
                tc.tile_pool(name="xpool", bufs=12) as xpool,
                tc.tile_pool(name="stage", bufs=4) as stage,
                tc.tile_pool(name="pps", bufs=6, space="PSUM") as pps,
            ):
                for s in "vkq":
                    w1sb = wpool.tile([128, 8, GCH], MMD, tag="w")
                    w2sb = wpool.tile([128, 8, GCH], MMD, tag="w")
                    nc.sync.dma_start(
                        w1sb[:], w1T[s][:].rearrange("(dc p) o -> p dc o", p=128)
                    )
                    nc.sync.dma_start(
                        w2sb[:], w2T[s][:].rearrange("(dc p) o -> p dc o", p=128)
                    )
                    if s != "v":
                        b1sb = persist.tile([128, 4], F32, tag=f"b1{s}")
                        b2sb = persist.tile([128, 4], F32, tag=f"b2{s}")
                        b1hsb = persist.tile([128, 4], F32, tag=f"b1h{s}")
                        nc.sync.dma_start(b1sb[:], bias_d[f"b1_{s}"][:])
                        nc.sync.dma_start(b2sb[:], bias_d[f"b2_{s}"][:])
                        nc.sync.dma_start(b1hsb[:], bias_d[f"b1h_{s}"][:])
                    else:
                        b1vr = persist.tile([1, GCH], MMD, tag="b1v")
                        b2vr = persist.tile([1, GCH], MMD, tag="b2v")
                        nc.sync.dma_start(b1vr[:], b1v_d[:])
                        nc.sync.dma_start(b2vr[:], b2v_d[:])

                    for t in range(4):  # 512-wide seq tiles
                        xts = []
                        for dc in range(8):
                            xt = xpool.tile([128, 512], MMD, tag="xt")
                            nc.sync.dma_start(
                                xt[:],
                                xT[s][dc * 128:(dc + 1) * 128,
                                      t * 512:(t + 1) * 512],
                            )
                            xts.append(xt)
                        for jh in range(2):
                            ps1 = [pps.tile([128, 512], F32, tag="pp",
                                            name=f"ps1_{i}")
                                   for i in range(2)]
                            ps2 = [pps.tile([128, 512], F32, tag="pp",
                                            name=f"ps2_{i}")
                                   for i in range(2)]
                            for dc in range(8):
                                for jj in range(2):
                                    j = jh * 2 + jj
                                    if s == "v":
                                        # seq on partitions: lhsT = x chunk
                                        nc.tensor.matmul(
                                            ps1[jj][:],
                                            xts[dc][:, j * 128:(j + 1) * 128],
                                            w1sb[:, dc, :],
                                            start=(dc == 0), stop=False,
                                        )
                                        nc.tensor.matmul(
                                            ps2[jj][:],
                                            xts[dc][:, j * 128:(j + 1) * 128],
                                            w2sb[:, dc, :],
                                            start=(dc == 0), stop=False,
                                        )
                                    else:
                                        # channels on partitions: lhsT = w chunk
                                        nc.tensor.matmul(
                                            ps1[jj][:],
                                            w1sb[:, dc, j * 128:(j + 1) * 128],
                                            xts[dc][:],
                                            start=(dc == 0), stop=(dc == 7),
                                        )
                                        nc.tensor.matmul(
                                            ps2[jj][:],
                                            w2sb[:, dc, j * 128:(j + 1) * 128],
                                            xts[dc][:],
                                            start=(dc == 0), stop=(dc == 7),
                                        )
                            for jj in range(2):
                                j = jh * 2 + jj
                                act = stage.tile([128, 512], F32, tag="act")
                                if s == "v":
                                    # fold the biases into the accumulation
                                    # (they vary along the free/channel dim)
                                    nc.tensor.matmul(
                                        ps1[jj][:], ones_r, b1vr[:],
                                        start=False, stop=True,
                                    )
                                    nc.tensor.matmul(
                                        ps2[jj][:], ones_r, b2vr[:],
                                        start=False, stop=True,
                                    )
                                    nc.scalar.activation(
                                        act[:], ps1[jj][:], ACTF.Tanh,
                                        scale=0.5,
                                    )
                                    u = stage.tile([128, 512], F32, tag="u")
                                    nc.vector.tensor_tensor(
                                        u[:], ps1[jj][:], act[:], ALU.mult
                                    )
                                    nc.vector.tensor_tensor(
                                        act[:], ps1[jj][:], u[:], ALU.add
                                    )
                                    nt_i = t * 4 + j
                                    nc.vector.tensor_tensor(
                                        v_sb[:, nt_i, :, 0:64],
                                        ps2[jj][:].rearrange(
                                            "p (h d) -> p h d", h=8
                                        ),
                                        act[:].rearrange(
                                            "p (h d) -> p h d", h=8
                                        ),
                                        ALU.mult,
                                    )
                                else:
                                    bias1 = b1sb[:, j:j + 1]
                                    bias2 = b2sb[:, j:j + 1]
                                    # act = tanh((A)/2), A = ps1 + b1
                                    nc.scalar.activation(
                                        act[:], ps1[jj][:], ACTF.Tanh,
                                        scale=0.5, bias=b1hsb[:, j:j + 1],
                                    )
                                    a_sb = stage.tile([128, 512], F32,
                                                      tag="u")
                                    nc.vector.tensor_scalar_add(
                                        a_sb[:], ps1[jj][:], bias1
                                    )
                                    # act = A*(1+tanh(A/2)) = 2*silu(A)
                                    nc.vector.scalar_tensor_tensor(
                                        act[:], act[:], 1.0, a_sb[:],
                                        op0=ALU.add, op1=ALU.mult,
                                    )
                                    dst = (qt_sb if s == "q" else kt_sb)[
                                        :, j, t * 512:(t + 1) * 512
                                    ]
                                    nc.vector.scalar_tensor_tensor(
                                        dst, ps2[jj][:], bias2, act[:],
                                        op0=ALU.add, op1=ALU.mult,
                                    )

            # ------------- Phase B+C: attention + output projection -------
            with (
                tc.tile_pool(name="scps", bufs=2, space="PSUM") as scps,
                tc.tile_pool(name="cxps", bufs=2, space="PSUM") as cxps,
                tc.tile_pool(name="bcps", bufs=2, space="PSUM") as bcps,
                tc.tile_pool(name="apool", bufs=6) as apool,
                tc.tile_pool(name="ctpool", bufs=2) as ctpool,
                tc.tile_pool(name="smalls", bufs=4) as smalls,
                tc.tile_pool(name="ostage", bufs=4) as ostage,
                tc.tile_pool(name="mpool", bufs=2) as mpool,
            ):
                for qg in range(4):
                    kcmax = kc_count(qg)
                    qsl = slice(qg * 512, (qg + 1) * 512)
                    ct_qg = ctpool.tile([128, 4, 512], MMD, tag="ct")

                    mtiles = None
                    if mask_mode == "general":
                        mtiles = []
                        mt_sb = mpool.tile([128, NT, 512], MMD, tag="mt")
                        for kc in range(kcmax):
                            nc.sync.dma_start(
                                mt_sb[:, kc, :],
                                m01T_d[kc * 128:(kc + 1) * 128, qsl],
                            )
                            mtiles.append(mt_sb[:, kc, :])

                    for pj in range(4):   # head pair: hl = 2*pj (+1)
                        ctx = [cxps.tile([128, 512], F32, tag="cx",
                                         name=f"ctx_{i}")
                               for i in range(2)]
                        for kc in range(kcmax):
                            ksl = slice(kc * 128, (kc + 1) * 128)
                            # diagonal chunks: only q >= off is unmasked
                            diag = mask_mode == "causal" and kc >= 4 * qg
                            off = 128 * (kc - 4 * qg) if diag else 0
                            w = 512 - off
                            sc2 = scps.tile([128, 1024], F32, tag="sc",
                                            name="sc2")
                            for par in range(2):
                                bp = par * 64
                                nc.tensor.matmul(
                                    sc2[:, par * 512 + off:(par + 1) * 512],
                                    kt_sb[bp:bp + 64, pj, ksl],
                                    qt_sb[bp:bp + 64, pj,
                                          qg * 512 + off:(qg + 1) * 512],
                                )
                            attn = apool.tile([128, 1024], MMD, tag="at")
                            sc_v = sc2[:].rearrange("p (a b) -> p a b", a=2)
                            at_v = attn[:].rearrange("p (a b) -> p a b", a=2)
                            nc.scalar.activation(at_v[:, :, off:],
                                                 sc_v[:, :, off:], ACTF.Exp)
                            if diag:
                                # only the leading 128 columns of the
                                # restricted range touch the triangle
                                nc.vector.tensor_tensor(
                                    at_v[:, :, off:off + 128],
                                    at_v[:, :, off:off + 128],
                                    pat_sb[:, kc - 4 * qg, off:off + 128]
                                    [:, None, :].to_broadcast([128, 2, 128]),
                                    ALU.mult,
                                )
                            elif mask_mode == "general":
                                for par in range(2):
                                    asl = attn[:, par * 512:(par + 1) * 512]
                                    nc.vector.tensor_tensor(
                                        asl, asl, mtiles[kc], ALU.mult,
                                    )
                            for par in range(2):
                                hl = 2 * pj + par
                                nc.tensor.matmul(
                                    ctx[par][0:65, off:512],
                                    v_sb[:, kc, hl, :],
                                    attn[:, par * 512 + off:(par + 1) * 512],
                                    start=(kc == 0),
                                    stop=(kc == kcmax - 1),
                                )
                        # snapshot ctx (incl. den row 64) to SBUF right
                        # away so the PSUM banks free for the next head pair;
                        # then batched reciprocal + PE broadcast off-PSUM.
                        cts = [smalls.tile([65, 512], F32, tag=f"cts{i}",
                                           name=f"cts{i}") for i in range(2)]
                        for par in range(2):
                            nc.vector.tensor_copy(cts[par][:], ctx[par][0:65, :])
                        den = smalls.tile([33, 512], F32, tag="den")
                        nc.gpsimd.memset(den[:], 1.0)
                        for par in range(2):
                            # read the den row straight from PSUM: runs on the
                            # scalar engine in parallel with the DVE cts copies
                            nc.scalar.activation(
                                den[32 * par:32 * par + 1, :],
                                ctx[par][64:65, :], ACTF.Identity,
                            )
                        recf = smalls.tile([33, 512], F32, tag="recf")
                        nc.vector.reciprocal_approx_fast(recf[:], den[:])
                        rec = smalls.tile([33, 512], MMD, tag="rec")
                        nc.vector.tensor_copy(rec[:], recf[:])
                        rec_rows = (rec[0:1, :], rec[32:33, :])
                        ones_rows = (ones_c[0:1, 0:64], ones_c[32:33, 0:64])
                        for par in range(2):
                            bc_ps = bcps.tile([128, 512], F32, tag="bc")
                            nc.tensor.matmul(
                                bc_ps[0:64, :], ones_rows[par],
                                rec_rows[par],
                            )
                            bc_sb = smalls.tile([64, 512], F32, tag="bcs")
                            nc.vector.tensor_copy(bc_sb[:], bc_ps[0:64, :])
                            bp = par * 64
                            nc.vector.tensor_tensor(
                                ct_qg[bp:bp + 64, pj, :],
                                cts[par][0:64, :], bc_sb[:], ALU.mult,
                            )

                    # ---- output projection for this q-group ----
                    for ns in range(4):
                        nt_i = qg * 4 + ns
                        nsl = slice(ns * 128, (ns + 1) * 128)
                        for oh in range(2):
                            po = bcps.tile([128, 512], F32, tag="bc")
                            for j in range(4):
                                nc.tensor.matmul(
                                    po[:],
                                    ct_qg[:, j, nsl],
                                    woT_sb[:, j, oh * 512:(oh + 1) * 512],
                                    start=(j == 0), stop=(j == 3),
                                )
                            ot = ostage.tile([128, 512], F32, tag="ot")
                            nc.vector.tensor_copy(ot[:], po[:])
                            nc.sync.dma_start(
                                pout_d[nt_i * 128:(nt_i + 1) * 128,
                                       oh * 512:(oh + 1) * 512],
                                ot[:],
                            )
    nc.compile()
    return nc


def _host_prepare(inputs):
    """Split the full problem into 8 per-core input maps + host-side info."""
    q = np.asarray(inputs["query"], dtype=np.float32)
    k = np.asarray(inputs["key"], dtype=np.float32)
    v = np.asarray(inputs["value"], dtype=np.float32)
    mask = np.asarray(inputs["mask"])
    w = {n: np.asarray(inputs[n], dtype=np.float32)
         for n in ("wq1", "wq2", "wk1", "wk2", "wv1", "wv2", "wo")}
    bias = {n: np.asarray(inputs[n], dtype=np.float32)
            for n in ("bq1", "bq2", "bk1", "bk2", "bv1", "bv2", "bo")}

    m = mask.reshape(S, S)
    if np.array_equal(m != 0, np.tril(np.ones((S, S), bool))):
        mask_mode = "causal"
    elif np.all(m != 0):
        mask_mode = "full"
    else:
        mask_mode = "general"

    pat = None
    m01T = None
    if mask_mode == "causal":
        kk = np.arange(128)[:, None]
        qq = np.arange(512)[None, :]
        pat = np.stack(
            [(kk + 128 * i <= qq).astype(np.float32) for i in range(4)], axis=1
        )  # [128, 4, 512]
        pat = np.ascontiguousarray(pat)
    elif mask_mode == "general":
        m01T = np.ascontiguousarray((m != 0).T.astype(np.float32))

    scale = 1.0 / np.sqrt(DK).astype(np.float32)

    if MM_DTYPE == "bf16":
        import ml_dtypes

        mmd_np = ml_dtypes.bfloat16
    else:
        mmd_np = np.float32

    def cvt(a):
        return np.ascontiguousarray(a).astype(mmd_np)

    in_maps = []
    for c in range(NCORES):
        b, g = divmod(c, 2)
        sl = slice(g * GCH, (g + 1) * GCH)
        im = {
            "xqT": cvt(q[b].T),
            "xkT": cvt(k[b].T),
            "xvT": cvt(v[b].T),
            "w1T_q": cvt(w["wq1"][sl].T),
            # fold the 1/sqrt(dk) score scale into the non-silu Q branch,
            # and 0.5 everywhere (silu computed as A*(1+tanh(A/2)) = 2*silu)
            "w2T_q": cvt(w["wq2"][sl].T * (scale * 0.5)),
            "w2T_k": cvt(w["wk2"][sl].T * 0.5),
            "w2T_v": cvt(w["wv2"][sl].T * 0.5),
            "w1T_k": cvt(w["wk1"][sl].T),
            "w1T_v": cvt(w["wv1"][sl].T),
            "b1_q": np.ascontiguousarray(bias["bq1"][sl].reshape(4, 128).T),
            "b1h_q": np.ascontiguousarray(
                (bias["bq1"][sl] * 0.5).reshape(4, 128).T),
            "b2_q": np.ascontiguousarray(
                (bias["bq2"][sl] * (scale * 0.5)).reshape(4, 128).T),
            "b1_k": np.ascontiguousarray(bias["bk1"][sl].reshape(4, 128).T),
            "b1h_k": np.ascontiguousarray(
                (bias["bk1"][sl] * 0.5).reshape(4, 128).T),
            "b2_k": np.ascontiguousarray(
                (bias["bk2"][sl] * 0.5).reshape(4, 128).T),
            "b1_v": cvt(bias["bv1"][sl].reshape(1, GCH)),
            "b2_v": cvt((bias["bv2"][sl] * 0.5).reshape(1, GCH)),
            "woT": cvt(
                w["wo"][:, sl].T.reshape(4, 128, D).transpose(1, 0, 2)),
        }
        if mask_mode == "causal":
            im["pat"] = cvt(pat)
        elif mask_mode == "general":
            im["m01T"] = cvt(m01T)
        in_maps.append(im)
    return mask_mode, in_maps, bias["bo"]


def kernel(**inputs):
    global LAST_RESULT
    mask_mode, in_maps, bo = _host_prepare(inputs)
    nc = build_program(mask_mode)

    import concourse.bass_utils as bu

    if TRACE:
        import types

        try:
            from trn_agent_boot.trn_boot import _ntff_profile_via_ctypes

            hook = _ntff_profile_via_ctypes("/opt/axon/libaxon_pjrt.so")
            m = types.ModuleType("antenv.axon_hooks")
            m.get_axon_ntff_profile_hook = lambda: hook
            import antenv  # noqa: F401

            sys.modules["antenv.axon_hooks"] = m
            bu.upload_artifacts = lambda d: "local://skipped"
        except Exception as e:
            print("profiling hook install failed:", e)

    res = bu.run_bass_kernel_spmd(
        nc, in_maps, core_ids=list(range(NCORES)),
        trace=TRACE, trace_cores=TRACE_CORES,
    )
    LAST_RESULT = res

    out = np.empty((B, S, D), dtype=np.float32)
    for b in range(B):
        out[b] = (res.results[2 * b]["pout"] + res.results[2 * b + 1]["pout"]
                  + bo[None, :])
    return out


# revision 27
# speedup vs baseline: 1.5023x; 1.0397x over previous
"""SwiGLU-projected causal MHA (B=4, S=2048, D=1024, H=16) on 8 TRN2 NeuronCores.

Baseline (572888 ns) restored from the original staged kernel.

Sharding: core c -> (batch b = c//2, head-group g = c%2).  Each core computes
the SwiGLU Q/K/V projections for its 512 output channels (= 8 heads) of its
batch, runs causal attention for those heads, and produces a partial output
projection (contraction over its 512 channels).  The host sums the two
partials per batch and adds the output bias.
"""
import sys

sys.path.insert(0, "/opt/trn_rl_repo")
import numpy as np

import concourse.bacc as bacc
import concourse.tile as tile
import concourse.mybir as mybir

B, S, D = 4, 2048, 1024
H, DK = 16, 64
NCORES = 8
GCH = 512          # channels per core (8 heads)
NT = S // 128      # 16 seq chunks
F32 = mybir.dt.float32
F32R = mybir.dt.float32r
ACTF = mybir.ActivationFunctionType
ALU = mybir.AluOpType

TRACE = False          # set by test.py for profiling runs
TRACE_CORES = None
LAST_RESULT = None     # BassKernelResults stash for test.py
MM_DTYPE = "bf16"      # "bf16" (fast weight load) or "f32r" (higher precision)


def build_program(mask_mode):
    """mask_mode: 'causal' (tril), 'full' (all ones), 'general' (arbitrary)."""
    MMD = mybir.dt.bfloat16 if MM_DTYPE == "bf16" else F32R
    nc = bacc.Bacc("TRN2", target_bir_lowering=False, debug=False)

    xT = {s: nc.dram_tensor(f"x{s}T", [D, S], MMD, kind="ExternalInput")
          for s in "qkv"}
    w1T = {s: nc.dram_tensor(f"w1T_{s}", [D, GCH], MMD, kind="ExternalInput")
           for s in "qkv"}
    w2T = {s: nc.dram_tensor(f"w2T_{s}", [D, GCH], MMD, kind="ExternalInput")
           for s in "qkv"}
    bias_d = {}
    for s in "qk":
        for bn in ("b1", "b2", "b1h"):
            bias_d[f"{bn}_{s}"] = nc.dram_tensor(f"{bn}_{s}", [128, 4], F32,
                                                 kind="ExternalInput")
    b1v_d = nc.dram_tensor("b1_v", [1, GCH], MMD, kind="ExternalInput")
    b2v_d = nc.dram_tensor("b2_v", [1, GCH], MMD, kind="ExternalInput")
    woT_d = nc.dram_tensor("woT", [128, 4, D], MMD, kind="ExternalInput")
    pat_d = m01T_d = None
    if mask_mode == "causal":
        pat_d = nc.dram_tensor("pat", [128, 4, 512], MMD, kind="ExternalInput")
    elif mask_mode == "general":
        m01T_d = nc.dram_tensor("m01T", [S, S], MMD, kind="ExternalInput")
    pout_d = nc.dram_tensor("pout", [S, D], F32, kind="ExternalOutput")

    def kc_count(qg):
        return 4 * qg + 4 if mask_mode == "causal" else NT

    with tile.TileContext(nc) as tc:
        with (
            tc.tile_pool(name="persist", bufs=1) as persist,
        ):
            qt_sb = persist.tile([128, 4, S], MMD, tag="qt")
            kt_sb = persist.tile([128, 4, S], MMD, tag="kt")
            v_sb = persist.tile([128, NT, 8, 65], MMD, tag="v")
            woT_sb = persist.tile([128, 4, D], MMD, tag="wo")
            nc.sync.dma_start(woT_sb[:], woT_d[:])
            onesf = persist.tile([33, 128], F32, tag="onesf")
            ones_c = persist.tile([33, 128], MMD, tag="ones_c")
            nc.any.memset(onesf[:], 1.0)
            nc.vector.tensor_copy(ones_c[:], onesf[:])
            ones_r = ones_c[0:1, :]
            onescol = persist.tile([128, 1], F32, tag="onescol")
            nc.any.memset(onescol[:], 1.0)
            nc.vector.tensor_copy(
                v_sb[:, :, :, 64:65],
                onescol[:, None, :].to_broadcast([128, NT, 8, 1]),
            )
            if mask_mode == "causal":
                pat_sb = persist.tile([128, 4, 512], MMD, tag="pat")
                nc.sync.dma_start(pat_sb[:], pat_d[:])

            # ---------------- Phase A: SwiGLU projections ----------------
            with (
                tc.tile_pool(name="wpool", bufs=4) as wpool,
                tc.tile_pool(name="xpool", bufs=12) as xpool,
                tc.tile_pool(name="stage", bufs=4) as stage,
                tc.tile_pool(name="pps", bufs=6, space="PSUM") as pps,
            ):
                for s in "vkq":
                    w1sb = wpool.tile([128, 8, GCH], MMD, tag="w")
                    w2sb = wpool.tile([128, 8, GCH], MMD, tag="w")
                    nc.sync.dma_start(
                        w1sb[:], w1T[s][:].rearrange("(dc p) o -> p dc o", p=128)
                    )
                    nc.sync.dma_start(
                        w2sb[:], w2T[s][:].rearrange("(dc p) o -> p dc o", p=128)
                    )
                    if s != "v":
                        b1sb = persist.tile([128, 4], F32, tag=f"b1{s}")
                        b2sb = persist.tile([128, 4], F32, tag=f"b2{s}")
                        b1hsb = persist.tile([128, 4], F32, tag=f"b1h{s}")
                        nc.sync.dma_start(b1sb[:], bias_d[f"b1_{s}"][:])
                        nc.sync.dma_start(b2sb[:], bias_d[f"b2_{s}"][:])
                        nc.sync.dma_start(b1hsb[:], bias_d[f"b1h_{s}"][:])
                    else:
                        b1vr = persist.tile([1, GCH], MMD, tag="b1v")
                        b2vr = persist.tile([1, GCH], MMD, tag="b2v")
                        nc.sync.dma_start(b1vr[:], b1v_d[:])
                        nc.sync.dma_start(b2vr[:], b2v_d[:])

                    for t in range(4):  # 512-wide seq tiles
                        xts = []
                        for dc in range(8):
                            xt = xpool.tile([128, 512], MMD, tag="xt")
                            nc.sync.dma_start(
                                xt[:],
                                xT[s][dc * 128:(dc + 1) * 128,
                                      t * 512:(t + 1) * 512],
                            )
                            xts.append(xt)
                        for jh in range(2):
                            ps1 = [pps.tile([128, 512], F32, tag="pp",
                                            name=f"ps1_{i}")
                                   for i in range(2)]
                            ps2 = [pps.tile([128, 512], F32, tag="pp",
                                            name=f"ps2_{i}")
                                   for i in range(2)]
                            for dc in range(8):
                                for jj in range(2):
                                    j = jh * 2 + jj
                                    if s == "v":
                                        # seq on partitions: lhsT = x chunk
                                        nc.tensor.matmul(
                                            ps1[jj][:],
                                            xts[dc][:, j * 128:(j + 1) * 128],
                                            w1sb[:, dc, :],
                                            start=(dc == 0), stop=False,
                                        )
                                        nc.tensor.matmul(
                                            ps2[jj][:],
                                            xts[dc][:, j * 128:(j + 1) * 128],
                                            w2sb[:, dc, :],
                                            start=(dc == 0), stop=False,
                                        )
                                    else:
                                        # channels on partitions: lhsT = w chunk
                                        nc.tensor.matmul(
                                            ps1[jj][:],
                                            w1sb[:, dc, j * 128:(j + 1) * 128],
                                            xts[dc][:],
                                            start=(dc == 0), stop=(dc == 7),
                                        )
                                        nc.tensor.matmul(
                                            ps2[jj][:],
                                            w2sb[:, dc, j * 128:(j + 1) * 128],
                                            xts[dc][:],
                                            start=(dc == 0), stop=(dc == 7),
                                        )
                            for jj in range(2):
                                j = jh * 2 + jj
                                act = stage.tile([128, 512], F32, tag="act")
                                if s == "v":
                                    # fold the biases into the accumulation
                                    # (they vary along the free/channel dim)
                                    nc.tensor.matmul(
                                        ps1[jj][:], ones_r, b1vr[:],
                                        start=False, stop=True,
                                    )
                                    nc.tensor.matmul(
                                        ps2[jj][:], ones_r, b2vr[:],
                                        start=False, stop=True,
                                    )
                                    nc.scalar.activation(
                                        act[:], ps1[jj][:], ACTF.Tanh,
                                        scale=0.5,
                                    )
                                    u = stage.tile([128, 512], F32, tag="u")
                                    nc.vector.tensor_tensor(
                                        u[:], ps1[jj][:], act[:], ALU.mult
                                    )
                                    nc.vector.tensor_tensor(
                                        act[:], ps1[jj][:], u[:], ALU.add
                                    )
                                    nt_i = t * 4 + j
                                    nc.vector.tensor_tensor(
                                        v_sb[:, nt_i, :, 0:64],
                                        ps2[jj][:].rearrange(
                                            "p (h d) -> p h d", h=8
                                        ),
                                        act[:].rearrange(
                                            "p (h d) -> p h d", h=8
                                        ),
                                        ALU.mult,
                                    )
                                else:
                                    bias1 = b1sb[:, j:j + 1]
                                    bias2 = b2sb[:, j:j + 1]
                                    # act = tanh((A)/2), A = ps1 + b1
                                    nc.scalar.activation(
                                        act[:], ps1[jj][:], ACTF.Tanh,
                                        scale=0.5, bias=b1hsb[:, j:j + 1],
                                    )
                                    a_sb = stage.tile([128, 512], F32,
                                                      tag="u")
                                    nc.vector.tensor_scalar_add(
                                        a_sb[:], ps1[jj][:], bias1
                                    )
                                    # act = A*(1+tanh(A/2)) = 2*silu(A)
                                    nc.vector.scalar_tensor_tensor(
                                        act[:], act[:], 1.0, a_sb[:],
                                        op0=ALU.add, op1=ALU.mult,
                                    )
                                    dst = (qt_sb if s == "q" else kt_sb)[
                                        :, j, t * 512:(t + 1) * 512
                                    ]
                                    nc.vector.scalar_tensor_tensor(
                                        dst, ps2[jj][:], bias2, act[:],
                                        op0=ALU.add, op1=ALU.mult,
                                    )

            # ------------- Phase B+C: attention + output projection -------
            with (
                tc.tile_pool(name="scps", bufs=2, space="PSUM") as scps,
                tc.tile_pool(name="cxps", bufs=2, space="PSUM") as cxps,
                tc.tile_pool(name="bcps", bufs=2, space="PSUM") as bcps,
                tc.tile_pool(name="apool", bufs=6) as apool,
                tc.tile_pool(name="ctpool", bufs=2) as ctpool,
                tc.tile_pool(name="smalls", bufs=4) as smalls,
                tc.tile_pool(name="ostage", bufs=4) as ostage,
                tc.tile_pool(name="mpool", bufs=2) as mpool,
            ):
                for qg in range(4):
                    kcmax = kc_count(qg)
                    qsl = slice(qg * 512, (qg + 1) * 512)
                    ct_qg = ctpool.tile([128, 4, 512], MMD, tag="ct")

                    mtiles = None
                    if mask_mode == "general":
                        mtiles = []
                        mt_sb = mpool.tile([128, NT, 512], MMD, tag="mt")
                        for kc in range(kcmax):
                            nc.sync.dma_start(
                                mt_sb[:, kc, :],
                                m01T_d[kc * 128:(kc + 1) * 128, qsl],
                            )
                            mtiles.append(mt_sb[:, kc, :])

                    for pj in range(4):   # head pair: hl = 2*pj (+1)
                        ctx = [cxps.tile([128, 512], F32, tag="cx",
                                         name=f"ctx_{i}")
                               for i in range(2)]
                        for kc in range(kcmax):
                            ksl = slice(kc * 128, (kc + 1) * 128)
                            # diagonal chunks: only q >= off is unmasked
                            diag = mask_mode == "causal" and kc >= 4 * qg
                            off = 128 * (kc - 4 * qg) if diag else 0
                            w = 512 - off
                            sc2 = scps.tile([128, 1024], F32, tag="sc",
                                            name="sc2")
                            for par in range(2):
                                bp = par * 64
                                nc.tensor.matmul(
                                    sc2[:, par * 512 + off:(par + 1) * 512],
                                    kt_sb[bp:bp + 64, pj, ksl],
                                    qt_sb[bp:bp + 64, pj,
                                          qg * 512 + off:(qg + 1) * 512],
                                )
                            attn = apool.tile([128, 1024], MMD, tag="at")
                            sc_v = sc2[:].rearrange("p (a b) -> p a b", a=2)
                            at_v = attn[:].rearrange("p (a b) -> p a b", a=2)
                            nc.scalar.activation(at_v[:, :, off:],
                                                 sc_v[:, :, off:], ACTF.Exp)
                            if diag:
                                # only the leading 128 columns of the
                                # restricted range touch the triangle
                                nc.vector.tensor_tensor(
                                    at_v[:, :, off:off + 128],
                                    at_v[:, :, off:off + 128],
                                    pat_sb[:, kc - 4 * qg, off:off + 128]
                                    [:, None, :].to_broadcast([128, 2, 128]),
                                    ALU.mult,
                                )
                            elif mask_mode == "general":
                                for par in range(2):
                                    asl = attn[:, par * 512:(par + 1) * 512]
                                    nc.vector.tensor_tensor(
                                        asl, asl, mtiles[kc], ALU.mult,
                                    )
                            for par in range(2):
                                hl = 2 * pj + par
                                nc.tensor.matmul(
                                    ctx[par][0:65, off:512],
                                    v_sb[:, kc, hl, :],
                                    attn[:, par * 512 + off:(par + 1) * 512],
                                    start=(kc == 0),
                                    stop=(kc == kcmax - 1),
                                )
                        # snapshot ctx (incl. den row 64) to SBUF right
                        # away so the PSUM banks free for the next head pair;
                        # then batched reciprocal + PE broadcast off-PSUM.
                        cts = [smalls.tile([65, 512], F32, tag=f"cts{i}",
                                           name=f"cts{i}") for i in range(2)]
                        for par in range(2):
                            nc.vector.tensor_copy(cts[par][:], ctx[par][0:65, :])
                        # one [33,512] batched reciprocal per head (the
                        # only approx_fast shape that is HW-safe in-kernel),
                        # each with its den row at partition 0 so the gpsimd
                        # broadcast can read it directly — no staging copies,
                        # and the PE never touches normalization
                        for par in range(2):
                            den = smalls.tile([33, 512], F32,
                                              tag=f"den{par}", name="den")
                            nc.gpsimd.memset(den[:], 1.0)
                            nc.scalar.activation(
                                den[0:1, :], ctx[par][64:65, :],
                                ACTF.Identity,
                            )
                            recf = smalls.tile([33, 512], F32,
                                               tag=f"recf{par}", name="recf")
                            nc.vector.reciprocal_approx_fast(recf[:], den[:])
                            bc_sb = smalls.tile([64, 512], F32, tag="bcs")
                            nc.gpsimd.partition_broadcast(bc_sb[:],
                                                          recf[0:1, :])
                            bp = par * 64
                            nc.vector.tensor_tensor(
                                ct_qg[bp:bp + 64, pj, :],
                                cts[par][0:64, :], bc_sb[:], ALU.mult,
                            )

                    # ---- output projection for this q-group ----
                    for ns in range(4):
                        nt_i = qg * 4 + ns
                        nsl = slice(ns * 128, (ns + 1) * 128)
                        for oh in range(2):
                            po = bcps.tile([128, 512], F32, tag="bc")
                            for j in range(4):
                                nc.tensor.matmul(
                                    po[:],
                                    ct_qg[:, j, nsl],
                                    woT_sb[:, j, oh * 512:(oh + 1) * 512],
                                    start=(j == 0), stop=(j == 3),
                                )
                            ot = ostage.tile([128, 512], F32, tag="ot")
                            nc.vector.tensor_copy(ot[:], po[:])
                            nc.sync.dma_start(
                                pout_d[nt_i * 128:(nt_i + 1) * 128,
                                       oh * 512:(oh + 1) * 512],
                                ot[:],
                            )
    nc.compile()
    return nc


def _host_prepare(inputs):
    """Split the full problem into 8 per-core input maps + host-side info."""
    q = np.asarray(inputs["query"], dtype=np.float32)
    k = np.asarray(inputs["key"], dtype=np.float32)
    v = np.asarray(inputs["value"], dtype=np.float32)
    mask = np.asarray(inputs["mask"])
    w = {n: np.asarray(inputs[n], dtype=np.float32)
         for n in ("wq1", "wq2", "wk1", "wk2", "wv1", "wv2", "wo")}
    bias = {n: np.asarray(inputs[n], dtype=np.float32)
            for n in ("bq1", "bq2", "bk1", "bk2", "bv1", "bv2", "bo")}

    m = mask.reshape(S, S)
    if np.array_equal(m != 0, np.tril(np.ones((S, S), bool))):
        mask_mode = "causal"
    elif np.all(m != 0):
        mask_mode = "full"
    else:
        mask_mode = "general"

    pat = None
    m01T = None
    if mask_mode == "causal":
        kk = np.arange(128)[:, None]
        qq = np.arange(512)[None, :]
        pat = np.stack(
            [(kk + 128 * i <= qq).astype(np.float32) for i in range(4)], axis=1
        )  # [128, 4, 512]
        pat = np.ascontiguousarray(pat)
    elif mask_mode == "general":
        m01T = np.ascontiguousarray((m != 0).T.astype(np.float32))

    scale = 1.0 / np.sqrt(DK).astype(np.float32)

    if MM_DTYPE == "bf16":
        import ml_dtypes

        mmd_np = ml_dtypes.bfloat16
    else:
        mmd_np = np.float32

    def cvt(a):
        return np.ascontiguousarray(a).astype(mmd_np)

    in_maps = []
    for c in range(NCORES):
        b, g = divmod(c, 2)
        sl = slice(g * GCH, (g + 1) * GCH)
        im = {
            "xqT": cvt(q[b].T),
            "xkT": cvt(k[b].T),
            "xvT": cvt(v[b].T),
            "w1T_q": cvt(w["wq1"][sl].T),
            # fold the 1/sqrt(dk) score scale into the non-silu Q branch,
            # and 0.5 everywhere (silu computed as A*(1+tanh(A/2)) = 2*silu)
            "w2T_q": cvt(w["wq2"][sl].T * (scale * 0.5)),
            "w2T_k": cvt(w["wk2"][sl].T * 0.5),
            "w2T_v": cvt(w["wv2"][sl].T * 0.5),
            "w1T_k": cvt(w["wk1"][sl].T),
            "w1T_v": cvt(w["wv1"][sl].T),
            "b1_q": np.ascontiguousarray(bias["bq1"][sl].reshape(4, 128).T),
            "b1h_q": np.ascontiguousarray(
                (bias["bq1"][sl] * 0.5).reshape(4, 128).T),
            "b2_q": np.ascontiguousarray(
                (bias["bq2"][sl] * (scale * 0.5)).reshape(4, 128).T),
            "b1_k": np.ascontiguousarray(bias["bk1"][sl].reshape(4, 128).T),
            "b1h_k": np.ascontiguousarray(
                (bias["bk1"][sl] * 0.5).reshape(4, 128).T),
            "b2_k": np.ascontiguousarray(
                (bias["bk2"][sl] * 0.5).reshape(4, 128).T),
            "b1_v": cvt(bias["bv1"][sl].reshape(1, GCH)),
            "b2_v": cvt((bias["bv2"][sl] * 0.5).reshape(1, GCH)),
            "woT": cvt(
                w["wo"][:, sl].T.reshape(4, 128, D).transpose(1, 0, 2)),
        }
        if mask_mode == "causal":
            im["pat"] = cvt(pat)
        elif mask_mode == "general":
            im["m01T"] = cvt(m01T)
        in_maps.append(im)
    return mask_mode, in_maps, bias["bo"]


def kernel(**inputs):
    global LAST_RESULT
    mask_mode, in_maps, bo = _host_prepare(inputs)
    nc = build_program(mask_mode)

    import concourse.bass_utils as bu

    if TRACE:
        import types

        try:
            from trn_agent_boot.trn_boot import _ntff_profile_via_ctypes

            hook = _ntff_profile_via_ctypes("/opt/axon/libaxon_pjrt.so")
            m = types.ModuleType("antenv.axon_hooks")
            m.get_axon_ntff_profile_hook = lambda: hook
            import antenv  # noqa: F401

            sys.modules["antenv.axon_hooks"] = m
            bu.upload_artifacts = lambda d: "local://skipped"
        except Exception as e:
            print("profiling hook install failed:", e)

    res = bu.run_bass_kernel_spmd(
        nc, in_maps, core_ids=list(range(NCORES)),
        trace=TRACE, trace_cores=TRACE_CORES,
    )
    LAST_RESULT = res

    out = np.empty((B, S, D), dtype=np.float32)
    for b in range(B):
        out[b] = (res.results[2 * b]["pout"] + res.results[2 * b + 1]["pout"]
                  + bo[None, :])
    return out
